# revision 13
# baseline (speedup 1.0000x reference)
"""Trainium2 Bass kernel for a 4-layer IndRNN (B=32, T=2048, I=256, H=512).

v3: 4-stream interleaved custom DVE ops (ANT_LSCAN4 / ANT_QREC4) with
2X_1PORT uop programs run the whole recurrence at 2 fp16 elem/cycle.
All 4 batches of a core are element-interleaved (a_t,b_t,c_t,d_t,...) in
one [128, 4T] fp16 stream per (layer, m-tile); one LSCAN4 + one QREC4
per group replaces the baseline's four pair ops at half the DVE time.

Math: per layer, with PSUM holding -xp (weights negated on host):
    l_t = w*l_{t-1} + (-xp_t)                     (LSCAN4; in-place)
    v_t = w*P_{t-1}; P_t = max(l_t, v_t); h_t = P_t - l_t   (QREC4)
which equals h_t = relu(xp_t + w*h_{t-1}), the IndRNN layer
(P - l == max(v - l, 0) saves an ALU stage -> 3 ops/elem, so two
elements fit the 8-stage datapath in 2x mode).

2x mode notes: rd1_en=1 with in1 := in0 forces the handler's TwoSrc perf
enable so only 2X_1PORT is reachable (2X_2PORT/4X would feed the uops a
port layout they can't drain -> engine hang); the uops consume SRC_1 into
dummy lanes. perf_max=1 is set on each instruction (byte-36[7:6]).

Sharding: data-parallel over batch, 4 batches (= 1 quad) per core.
"""

import numpy as np

from concourse import dve_ops
from concourse.dve_spec import Spec, Src0, C0, relu as sp_relu
from concourse.dve_uop import (
    AluInp,
    AluOp,
    DelayInp,
    DveOpSpec,
    ENABLE,
    InpSel,
    OutPath,
    OutSel,
    Trigger,
    UopConfig,
)

_REV = "r3"

# lanes: X=SRC_0(LO), W=CONST_0, XH=SRC_0_HI, HL=LO-result carry (2x),
# D1/D2 = dummy sinks for SRC_1/SRC_1_HI (consumed, never read)
_L_X, _L_W, _L_XH, _L_HL, _L_Z = 0, 1, 2, 3, 4
_L_D1, _L_D2 = 4, 5

_PD = AluInp.PREV_DELAY_0  # + lane id
_PA = AluInp.PREV_ALU_OUT


def _seed() -> UopConfig:
    """Zero the a-flops at stages 1..5 (superset of both ops' state flops)."""
    u = UopConfig()
    u.enable_input(InpSel.ZERO, _L_Z + 1)
    u.require_inp0 = 0
    u.repeat_count = 2
    u.trigger = (Trigger.COUNT, Trigger.NONE, Trigger.NONE)
    u.next_uop = (1, 0, 0)
    dp = u.datapath_config
    for k in range(5):
        dp[k].pass_through_delay(_L_Z)
    for k in range(1, 6):
        b = dp[k]
        b.op = AluOp.BYPASS
        b.alu_src0 = _PD + _L_Z
        b.alu_src1 = b.alu_src0
        b.alu_out_enable = ENABLE
        b.alu_out_a_enable = ENABLE
    return u


def _steady_base(next_idx: int, two_x: bool) -> UopConfig:
    u = UopConfig()
    u.enable_input(InpSel.SRC_0, _L_X + 1)
    u.enable_input(InpSel.CONST_0, _L_W + 1)
    u.enable_input(InpSel.SRC_1, _L_D1 + 1)
    if two_x:
        u.enable_input(InpSel.SRC_0_HI, _L_XH + 1)
        u.enable_input(InpSel.SRC_1_HI, _L_D2 + 1)
    u.require_inp0 = 1
    u.require_inp1 = 1
    u.repeat_count = 1
    u.trigger = (Trigger.SRC_TENSOR_DONE, Trigger.COUNT, Trigger.NONE)
    u.next_uop = (0, next_idx, 0)
    return u


def _chain_alu(u: UopConfig, from_blk: int):
    for k in range(from_blk, 8):
        u.datapath_config[k].pass_through_alu()
    u.enable_output(OutSel.ALU_OUT, OutPath.WR0_LO)


def _lscan_1x(mul: int, state: int, next_idx: int) -> UopConfig:
    u = _steady_base(next_idx, two_x=False)
    dp = u.datapath_config
    for k in range(mul):
        dp[k].pass_through_delay(_L_X, _L_W)
    dp[mul].enable_alu(AluOp.MULTIPLY, _PD + _L_W, AluInp.NEXT_ALU_OUT_A)
    dp[mul].pass_through_delay(_L_X)
    add = dp[state].enable_alu(AluOp.ADD, _PA, _PD + _L_X)
    add.alu_out_a_enable = ENABLE
    _chain_alu(u, state + 1)
    return u


def _lscan_2x(lo_mul: int, next_idx: int) -> UopConfig:
    u = _steady_base(next_idx, two_x=True)
    dp = u.datapath_config
    m0, a0 = lo_mul, lo_mul + 1
    m1, a1 = lo_mul + 2, lo_mul + 3
    for k in range(m0):
        dp[k].pass_through_delay(_L_X, _L_W, _L_XH)
    dp[m0].enable_alu(AluOp.MULTIPLY, _PD + _L_W, AluInp.NEXT_ALU_OUT_A)
    dp[m0].pass_through_delay(_L_X, _L_W, _L_XH)
    addlo = dp[a0].enable_alu(AluOp.ADD, _PA, _PD + _L_X)
    addlo.alu_out_a_enable = ENABLE
    addlo.pass_through_delay(_L_W, _L_XH)
    dp[m1].enable_alu(AluOp.MULTIPLY, _PD + _L_W, AluInp.NEXT_ALU_OUT_A)
    dp[m1].enable_delay_from_src(DelayInp.PREV_ALU_OUT, _L_HL)
    dp[m1].pass_through_delay(_L_XH)
    addhi = dp[a1].enable_alu(AluOp.ADD, _PA, _PD + _L_XH)
    addhi.alu_out_a_enable = ENABLE
    addhi.pass_through_delay(_L_HL)
    for k in range(a1 + 1, 8):
        dp[k].pass_through_alu()
        dp[k].pass_through_delay(_L_HL)
    u.enable_output(OutSel(int(OutSel.DELAY_0) + _L_HL), OutPath.WR0_LO)
    u.enable_output(OutSel.ALU_OUT, OutPath.WR0_HI)
    return u


def _qrec_1x(mul: int, next_idx: int) -> UopConfig:
    u = _steady_base(next_idx, two_x=False)
    dp = u.datapath_config
    for k in range(mul):
        dp[k].pass_through_delay(_L_X, _L_W)
    dp[mul].enable_alu(AluOp.MULTIPLY, _PD + _L_W, AluInp.NEXT_ALU_OUT_A)
    dp[mul].pass_through_delay(_L_X)
    pmax = dp[mul + 1].enable_alu(AluOp.MAX, _PA, _PD + _L_X)
    pmax.alu_out_a_enable = ENABLE
    pmax.pass_through_delay(_L_X)
    dp[mul + 2].enable_alu(AluOp.SUBTRACT, _PA, _PD + _L_X)
    _chain_alu(u, mul + 3)
    if mul + 1 == 6:
        u.accum_enabled = 1
    return u


def _qrec_2x(lo_mul: int, next_idx: int) -> UopConfig:
    u = _steady_base(next_idx, two_x=True)
    dp = u.datapath_config
    m = lo_mul
    for k in range(m):
        dp[k].pass_through_delay(_L_X, _L_W, _L_XH)
    dp[m].enable_alu(AluOp.MULTIPLY, _PD + _L_W, AluInp.NEXT_ALU_OUT_A)
    dp[m].pass_through_delay(_L_X, _L_W, _L_XH)
    pmax = dp[m + 1].enable_alu(AluOp.MAX, _PA, _PD + _L_X)
    pmax.alu_out_a_enable = ENABLE
    pmax.pass_through_delay(_L_X, _L_W, _L_XH)
    dp[m + 2].enable_alu(AluOp.SUBTRACT, _PA, _PD + _L_X)
    dp[m + 2].pass_through_delay(_L_W, _L_XH)
    dp[m + 3].enable_alu(AluOp.MULTIPLY, _PD + _L_W, AluInp.NEXT_ALU_OUT_A)
    dp[m + 3].enable_delay_from_src(DelayInp.PREV_ALU_OUT, _L_HL)
    dp[m + 3].pass_through_delay(_L_XH)
    pmaxh = dp[m + 4].enable_alu(AluOp.MAX, _PA, _PD + _L_XH)
    pmaxh.alu_out_a_enable = ENABLE
    pmaxh.pass_through_delay(_L_XH, _L_HL)
    dp[m + 5].enable_alu(AluOp.SUBTRACT, _PA, _PD + _L_XH)
    dp[m + 5].pass_through_delay(_L_HL)
    for k in range(m + 6, 8):
        dp[k].pass_through_alu()
        dp[k].pass_through_delay(_L_HL)
    u.enable_output(OutSel(int(OutSel.DELAY_0) + _L_HL), OutPath.WR0_LO)
    u.enable_output(OutSel.ALU_OUT, OutPath.WR0_HI)
    return u


def _lscan4_uops():
    # state flops: A@1, B@3, C@2, D@4
    return [
        _seed(),
        _lscan_1x(0, 1, next_idx=2),
        _lscan_1x(2, 3, next_idx=3),
        _lscan_1x(1, 2, next_idx=4),
        _lscan_1x(3, 4, next_idx=1),
    ]


def _lscan4_uops_2x():
    return [_seed(), _lscan_2x(0, next_idx=2), _lscan_2x(1, next_idx=1),
            UopConfig(), UopConfig()]


def _qrec4_uops():
    # state flops: A@1, B@4, C@2, D@5
    return [
        _seed(),
        _qrec_1x(0, next_idx=2),
        _qrec_1x(3, next_idx=3),
        _qrec_1x(1, next_idx=4),
        _qrec_1x(4, next_idx=1),
    ]


def _qrec4_uops_2x():
    return [_seed(), _qrec_2x(0, next_idx=2), _qrec_2x(1, next_idx=1),
            UopConfig(), UopConfig()]


def _ref_lscan4(in0, in1, c0, c1, c2):
    x = np.asarray(in0, np.float32).reshape(in0.shape[0], -1, 4)
    w = np.asarray(c0, np.float32).reshape(-1)
    out = np.empty_like(x)
    for s in range(4):
        acc = np.zeros_like(w)
        for t in range(x.shape[1]):
            acc = w * acc + x[:, t, s]
            out[:, t, s] = acc
    return out.reshape(in0.shape)


def _ref_qrec4(in0, in1, c0, c1, c2):
    el = np.asarray(in0, np.float32).reshape(in0.shape[0], -1, 4)
    w = np.asarray(c0, np.float32).reshape(-1)
    out = np.empty_like(el)
    for s in range(4):
        P_ = np.zeros_like(w)
        for t in range(el.shape[1]):
            v = w * P_
            P_ = np.maximum(el[:, t, s], v)
            out[:, t, s] = P_ - el[:, t, s]
    return out.reshape(in0.shape)


class _HandOp:
    """Duck-typed DveOp with hand-built uop programs (bypasses lower())."""

    def __init__(self, name, spec, uops, uops_2x, perf_max):
        self.name = name
        self.spec = spec
        self.subdim = False
        self._uops = uops
        self._uops_2x = uops_2x
        self._perf_max = perf_max
        self._cache = {}

    def compile(self, ver):
        assert ver == "v3", f"hand-built uops are v3-only, got {ver}"
        if ver not in self._cache:
            s = DveOpSpec(
                name=self.name,
                opcode=dve_ops.get_dve_sub_opcode(self.name),
                uops=self._uops,
                uops_2x=self._uops_2x,
                perf_max=self._perf_max,
                rd1_en=True,
            )
            s.validate(ver)
            self._cache[ver] = s
        return self._cache[ver]


def _register():
    import sys, types

    modname = f"ant_irnn4_ops_{_REV}"
    mod = sys.modules.get(modname)
    if mod is not None:
        return mod.LSCAN4, mod.QREC4
    spec1 = Spec(body=sp_relu(Src0 * C0), reference=_ref_lscan4)
    spec2 = Spec(body=sp_relu(Src0 * C0), reference=_ref_qrec4)
    ls = _HandOp(f"ANT_LSCAN4_{_REV}", spec1, _lscan4_uops(),
                 _lscan4_uops_2x(), perf_max=1)
    qr = _HandOp(f"ANT_QREC4_{_REV}", spec2, _qrec4_uops(),
                 _qrec4_uops_2x(), perf_max=1)
    base = max(dve_ops._SUB_OPCODE_FOR_NAME.values())
    dve_ops._SUB_OPCODE_FOR_NAME[ls.name] = base + 1
    dve_ops._SUB_OPCODE_FOR_NAME[qr.name] = base + 2
    assert max(dve_ops._SUB_OPCODE_FOR_NAME.values()) < 0x20
    dve_ops.OPS.append(ls)
    dve_ops.OPS.append(qr)
    dve_ops.CUSTOM_DVE_SPECS[ls.name] = ls.spec
    dve_ops.CUSTOM_DVE_SPECS[qr.name] = qr.spec
    mod = types.ModuleType(modname)
    mod.LSCAN4, mod.QREC4 = ls, qr
    sys.modules[modname] = mod
    return ls, qr


LSCAN4, QREC4 = _register()

# --- kernel ---

from contextlib import ExitStack

import concourse.bass as bass
import concourse.tile as tile
from concourse import mybir
from concourse.bass_utils import run_bass_kernel_spmd


dt = mybir.dt
Act = mybir.ActivationFunctionType

B, T, I, H, L = 32, 2048, 256, 512, 4
NCORES = 8
BLOC = B // NCORES          # 4 batches per core = one interleave quad
P = 128
TCH = 512
M4 = H // P
KI = I // P
QT = 4 * T                  # quad-interleaved stream length


def build(include_bias=False):
    nc = bass.Bass("TRN2", target_bir_lowering=False, debug=False,
                   num_devices=NCORES)
    xT_d = nc.dram_tensor("xT", [BLOC, I, T], dt.float16, kind="ExternalInput").ap()
    w0_d = nc.dram_tensor("w0tn", [I, H], dt.float16, kind="ExternalInput").ap()
    ws_d = nc.dram_tensor("wstn", [L - 1, H, H], dt.float16, kind="ExternalInput").ap()
    wq_d = nc.dram_tensor("wq", [P, L * M4], dt.float32, kind="ExternalInput").ap()
    bias_d = nc.dram_tensor("biasn", [L, 1, H], dt.float16, kind="ExternalInput").ap()
    # quad-interleaved output: [H, (t b)] -- host de-interleaves
    out_d = nc.dram_tensor("out", [H, QT], dt.float16,
                           kind="ExternalOutput").ap()

    with tile.TileContext(nc) as tc, ExitStack() as ctx:
        wpool = ctx.enter_context(tc.tile_pool(name="weights", bufs=1))
        xpool = ctx.enter_context(tc.tile_pool(name="xin", bufs=1))
        spool = ctx.enter_context(tc.tile_pool(name="stage", bufs=1))
        psum = ctx.enter_context(tc.tile_pool(name="psum", bufs=2, space="PSUM"))

        # ---- persistent weights ----
        wq_dmas, crit_dmas, late_dmas = [], [], []
        wqall = wpool.tile([P, L * M4], dt.float32, tag="wqall")
        wq_dmas.append(nc.gpsimd.dma_start(out=wqall[:], in_=wq_d))
        wq = [[wqall[:, (l * M4 + m):(l * M4 + m) + 1] for m in range(M4)]
              for l in range(L)]
        wt = [[] for _ in range(L)]
        for k in range(KI):
            w = wpool.tile([P, H], dt.float16, tag=f"w0{k}")
            crit_dmas.append(nc.gpsimd.dma_start(
                out=w[:], in_=w0_d[k * P:(k + 1) * P, :]))
            wt[0].append(w)
        # input tiles; batch 0 split into 4 chunks so the pipeline can start
        # on chunk 0, batches 1-3 full-tile afterwards
        xin = [[] for _ in range(BLOC)]
        xin_dmas = [[] for _ in range(BLOC)]
        for b in range(BLOC):
            for k in range(KI):
                xt = xpool.tile([P, T], dt.float16, tag=f"x{b}{k}")
                xin[b].append(xt)
        for c in range(4):
            cs = slice(c * TCH, (c + 1) * TCH)
            for b in (0, 1):
                for k in range(KI):
                    d = nc.gpsimd.dma_start(out=xin[b][k][:, cs],
                                            in_=xT_d[b, k * P:(k + 1) * P, cs])
                    crit_dmas.append(d)
                    xin_dmas[b].append((c, d))
        for b in (2, 3):
            for k in range(KI):
                d = nc.gpsimd.dma_start(out=xin[b][k][:],
                                        in_=xT_d[b, k * P:(k + 1) * P, :])
                late_dmas.append(d)
                xin_dmas[b].append((-1, d))
        ws_dmas = []
        for l in range(1, L):
            for k in range(M4):
                w = wpool.tile([P, H], dt.float16, tag=f"w{l}{k}")
                d = nc.gpsimd.dma_start(
                    out=w[:], in_=ws_d[l - 1, k * P:(k + 1) * P, :])
                ws_dmas.append(d)
                wt[l].append(w)
        bias = None
        if include_bias:
            bias = []
            for l in range(L):
                bt = wpool.tile([1, H], dt.float16, tag=f"b{l}")
                ws_dmas.append(nc.gpsimd.dma_start(out=bt[:],
                                                   in_=bias_d[l, :, :]))
                bias.append(bt)
            ones = wpool.tile([1, TCH], dt.float16, tag="ones")
            nc.gpsimd.memset(ones[:], 1.0)

        # ---- absorber machinery (per-engine pinned chains) ----
        scr_v = wpool.tile([P, 160], dt.float32, tag="scr_v")
        scr_a = wpool.tile([P, 160], dt.float32, tag="scr_a")
        state = {"V": [None, 0], "A": [None, 0], "PE": [None]}

        def absorb(eng, dep=None):
            if eng == "V":
                prev, k = state[eng]
                i = nc.vector.tensor_copy(scr_v[:, k:k + 1], wq[0][0][:])
            elif eng == "A":
                prev, k = state[eng]
                i = nc.scalar.activation(scr_a[:, k:k + 1], wq[0][0][:],
                                         Act.Copy)
            else:
                prev = state[eng][0]
                i = nc.tensor.ldweights(weights=wt[0][0][:, 0:P])
            if prev is not None:
                bass._add_dep_helper(i.ins, prev.ins, sync=False, reason="chain")
            if dep is not None:
                bass._add_dep_helper(i.ins, dep.ins, sync=True, reason="absorb")
            if eng == "PE":
                state[eng] = [i]
            else:
                state[eng] = [i, (state[eng][1] + 1) % 160]
            return i

        def pin(real, eng):
            prev = state[eng][0]
            if prev is not None:
                bass._add_dep_helper(real.ins, prev.ins, sync=False, reason="pin")
            state[eng][0] = real
            return real

        # warm-up: junk compute while input DMAs land (p-state ramp)
        jw = wpool.tile([P, P], dt.float16, tag="jw")
        jx = wpool.tile([P, 2048], dt.float16, tag="jx")
        jo = wpool.tile([P, 2048], dt.float16, tag="jo")
        jq = wpool.tile([P, 1], dt.float32, tag="jq")
        nc.vector.memset(jw[:, 0:1], 0.125)
        nc.vector.memset(jx[:, 0:1], 0.125)
        nc.vector.memset(jq[:], 0.125)
        for s in range(2):
            jp = psum.tile([P, T], dt.float32, tag="xp")
            for r in range(12):
                pin(nc.tensor.matmul(jp[:, 0:TCH], lhsT=jw[:],
                                     rhs=jx[:, 0:TCH],
                                     start=True, stop=True), "PE")
        for r in range(3):
            ji = nc.vector._custom_dve(LSCAN4, out=jo[:], in0=jx[:],
                                       in1=jx[:], s0=jq[:])
            ji.ins.perf_max = 1
            pin(ji, "V")

        # engine init
        for eng in ("V", "A"):
            absorb(eng)
            for d in wq_dmas:
                absorb(eng, d)
        for k in range(KI):
            pin(nc.tensor.ldweights(weights=wt[0][k][:, 0:P]), "PE")
        if include_bias:
            for l in range(L):
                pin(nc.tensor.ldweights(weights=bias[l][:, 0:P]), "PE")
            pin(nc.tensor.ldweights(weights=ones[:, 0:P]), "PE")

        # ---- main loop ----
        NXSB, NHP = 2, 7
        hp_ring = []        # (l, m) keys
        hp_by_key = {}
        hp_readers = {}     # key -> last matmul reading it
        xsb_readers = [None] * NXSB   # ring idx -> last QREC4 reader
        psum_readers = [None, None]   # psum slot -> ACT copy that read it
        xsb_i = 0
        psum_i = 0
        hp_i = 0
        last_qrec = None
        out_tiles = []  # layer-3 output tiles; fake end-of-trace readers keep
                        # them live so the allocator never hands a DMA-read
                        # buffer to a later tile (a DMA-completion release dep
                        # would exceed walrus's 1-wait budget)

        for l in range(L):
            kprev = KI if l == 0 else M4
            if l > 0:
                for k in range(M4):
                    pin(nc.tensor.ldweights(weights=wt[l][k][:, 0:P]), "PE")
                # PE absorbs the final QREC4 of layer l-1 (implies all of
                # layer l-1's h tiles via V-queue order)
                absorb("PE", last_qrec)
            else:
                for b in (2, 3):
                    for c, d in xin_dmas[b]:
                        if c < 0:
                            absorb("PE", d)
            for m in range(M4):
                xsb = spool.tile([P, QT], dt.float16, tag="xsb", bufs=NXSB)
                xsb_slot = xsb_i % NXSB
                xsb_i += 1
                last_cp = None
                for b in range(BLOC):
                    xp = psum.tile([P, T], dt.float32, tag="xp")
                    slot = psum_i % 2
                    psum_i += 1
                    old_rd = psum_readers[slot]
                    if old_rd is not None:
                        absorb("PE", old_rd)
                    last_mm = None
                    for n in range(T // TCH):
                        ns = slice(n * TCH, (n + 1) * TCH)
                        if l == 0:
                            for c, d in xin_dmas[b]:
                                if c == n:
                                    absorb("PE", d)
                        for k in range(kprev):
                            if l == 0:
                                rhs = xin[b][k][:, ns]
                            else:
                                hsrc = hp_by_key[(l - 1, k)]
                                rhs = hsrc[:, 4 * n * TCH + b:
                                           4 * (n + 1) * TCH:4]
                            last_mm = nc.tensor.matmul(
                                xp[:, ns], lhsT=wt[l][k][:, m * P:(m + 1) * P],
                                rhs=rhs, start=(k == 0),
                                stop=(k == kprev - 1 and not include_bias))
                            pin(last_mm, "PE")
                            if l > 0:
                                hp_readers[(l - 1, k)] = last_mm
                        if include_bias:
                            last_mm = pin(nc.tensor.matmul(
                                xp[:, ns], lhsT=bias[l][:, m * P:(m + 1) * P],
                                rhs=ones[:, :], start=False, stop=True), "PE")
                    # ACT: de-stride copy PSUM fp32 -> xsb quad slice b (fp16)
                    old_x = xsb_readers[xsb_slot]
                    if b == 0 and old_x is not None:
                        absorb("A", old_x)
                    absorb("A", last_mm)
                    cp = pin(nc.scalar.activation(
                        xsb[:].rearrange("p (t s) -> p s t", s=4)[
                            :, b:b + 1, :],
                        xp[:].rearrange("p (o t) -> p o t", o=1),
                        Act.Copy), "A")
                    psum_readers[slot] = cp
                    last_cp = cp
                # DVE: LSCAN4 in-place on xsb, then QREC4 -> hp tile
                absorb("V", last_cp)
                ls = nc.vector._custom_dve(LSCAN4, out=xsb[:], in0=xsb[:],
                                           in1=xsb[:], s0=wq[l][m][:])
                ls.ins.perf_max = 1
                pin(ls, "V")
                hpt = spool.tile([P, QT], dt.float16, tag="hp", bufs=NHP)
                if len(hp_ring) >= NHP:
                    old_key = hp_ring[hp_i % NHP]
                    rd = hp_readers.pop(old_key, None)
                    if rd is not None:
                        absorb("V", rd)
                if len(hp_ring) < NHP:
                    hp_ring.append((l, m))
                else:
                    hp_ring[hp_i % NHP] = (l, m)
                hp_i += 1
                hp_by_key[(l, m)] = hpt
                qr = nc.vector._custom_dve(QREC4, out=hpt[:], in0=xsb[:],
                                           in1=xsb[:], s0=wq[l][m][:])
                qr.ins.perf_max = 1
                pin(qr, "V")
                xsb_readers[xsb_slot] = qr
                last_qrec = qr
                if l == L - 1:
                    # 2 chunks per tile: 8 sync-queue DMAs total (the 9th
                    # would pick up a flow-control wait and trip walrus's
                    # 1-wait budget)
                    for c in range(2):
                        cs = slice(c * (QT // 2), (c + 1) * (QT // 2))
                        nc.sync.dma_start(
                            out=out_d[m * P:(m + 1) * P, cs],
                            in_=hpt[:, cs])
                    out_tiles.append(hpt)

        # fake readers: pin the DMA'd tiles live until end-of-trace
        for ht in out_tiles:
            pin(nc.vector.tensor_copy(scr_v[:, state["V"][1]:state["V"][1] + 1],
                                      ht[:, 0:1]), "V")
            state["V"][1] = (state["V"][1] + 1) % 160

        # ---- tail pre-drains ----
        tail_deps = [i for i in nc.inst_map.values()
                     if type(i).__name__ == "InstDMACopy"]
        snap = list(nc.inst_map.values())
        for eng in ("DVE", "Activation", "PE"):
            last_e = [i for i in snap
                      if str(getattr(i, "engine", "")).endswith(eng)]
            if last_e:
                tail_deps.append(last_e[-1])
        for depi in tail_deps:
            dr = nc.sync.drain(fusable=False)
            bass._add_dep_helper(dr.ins, depi, sync=True,
                                 reason="tail pre-drain absorber")
    assert mybir.codegen_inst_isa_subclasses(nc)
    _assert_wait_budget(nc)
    return nc


_MULTI_WAIT_OK = {"InstDrain",
                  "InstEventSemaphore", "InstUnconditionalBranch",
                  "InstRegisterMove", "InstISA", "InstTensorLoad",
                  "InstTensorSave"}


def _assert_wait_budget(nc):
    bad = []
    for name, inst in nc.inst_map.items():
        ty = type(inst).__name__
        w = inst.sync_info.on_wait if inst.sync_info else []
        if ty == "InstCustomDveAnt":
            fw = [x for x in w if not x.ant_name.startswith("DVE")]
            if fw:
                bad.append((name, ty, [f"{x.ant_name}>={x.wait_value}"
                                       for x in fw]))
            continue
        if ty in _MULTI_WAIT_OK:
            continue
        if len(w) > 1:
            bad.append((name, ty,
                        [f"{x.ant_name}>={x.wait_value}" for x in w]))
    if bad:
        raise RuntimeError(
            f"{len(bad)} instructions exceed the sync-wait budget, "
            f"first few: {bad[:6]}")


def _prep_core_inputs(Input, W0, Ws, bs, whs, core):
    bsl = slice(core * BLOC, (core + 1) * BLOC)
    return {
        "xT": np.ascontiguousarray(
            Input[bsl].transpose(0, 2, 1)).astype(np.float16),
        "w0tn": np.ascontiguousarray(-W0.T).astype(np.float16),
        "wstn": np.ascontiguousarray(-Ws.transpose(0, 2, 1)).astype(np.float16),
        "wq": np.ascontiguousarray(
            whs.astype(np.float32).reshape(L, M4, P).transpose(2, 0, 1)
            .reshape(P, L * M4)),
        "biasn": np.ascontiguousarray(-bs[:, None, :]).astype(np.float16),
    }


def kernel(Input, W0, Ws, bs, whs):
    include_bias = bool(np.any(bs != 0))
    nc = build(include_bias=include_bias)
    in_maps = [_prep_core_inputs(Input, W0, Ws, bs, whs, r)
               for r in range(NCORES)]
    res = run_bass_kernel_spmd(nc, in_maps, core_ids=list(range(NCORES)))
    parts = []
    for r in range(NCORES):
        o = res.results[r]["out"]  # [H, 4T] quad-interleaved
        o = o.reshape(H, T, BLOC).transpose(2, 0, 1)  # [BLOC, H, T]
        parts.append(o)
    full = np.concatenate(parts, axis=0)  # [B, H, T]
    return np.ascontiguousarray(full.transpose(0, 2, 1)).astype(np.float32)


# revision 17
# speedup vs baseline: 1.7643x; 1.7643x over previous
"""Trainium2 Bass kernel for a 4-layer IndRNN (B=32, T=2048, I=256, H=512).

v3: 4-stream interleaved custom DVE ops (ANT_LSCAN4 / ANT_QREC4) with
2X_1PORT uop programs run the whole recurrence at 2 fp16 elem/cycle.
All 4 batches of a core are element-interleaved (a_t,b_t,c_t,d_t,...) in
one [128, 4T] fp16 stream per (layer, m-tile); one LSCAN4 + one QREC4
per group replaces the baseline's four pair ops at half the DVE time.

Math: per layer, with PSUM holding -xp (weights negated on host):
    l_t = w*l_{t-1} + (-xp_t)                     (LSCAN4; in-place)
    v_t = w*P_{t-1}; P_t = max(l_t, v_t); h_t = P_t - l_t   (QREC4)
which equals h_t = relu(xp_t + w*h_{t-1}), the IndRNN layer
(P - l == max(v - l, 0) saves an ALU stage -> 3 ops/elem, so two
elements fit the 8-stage datapath in 2x mode).

2x mode notes: rd1_en=1 with in1 := in0 forces the handler's TwoSrc perf
enable so only 2X_1PORT is reachable (2X_2PORT/4X would feed the uops a
port layout they can't drain -> engine hang); the uops consume SRC_1 into
dummy lanes. perf_max=1 is set on each instruction (byte-36[7:6]).

Sharding: data-parallel over batch, 4 batches (= 1 quad) per core.
"""

import numpy as np

from concourse import dve_ops
from concourse.dve_spec import Spec, Src0, C0, relu as sp_relu
from concourse.dve_uop import (
    AluInp,
    AluOp,
    DelayInp,
    DveOpSpec,
    ENABLE,
    InpSel,
    OutPath,
    OutSel,
    Trigger,
    UopConfig,
)

_REV = "r3"

# lanes: X=SRC_0(LO), W=CONST_0, XH=SRC_0_HI, HL=LO-result carry (2x),
# D1/D2 = dummy sinks for SRC_1/SRC_1_HI (consumed, never read)
_L_X, _L_W, _L_XH, _L_HL, _L_Z = 0, 1, 2, 3, 4
_L_D1, _L_D2 = 4, 5

_PD = AluInp.PREV_DELAY_0  # + lane id
_PA = AluInp.PREV_ALU_OUT


def _seed() -> UopConfig:
    """Zero the a-flops at stages 1..5 (superset of both ops' state flops)."""
    u = UopConfig()
    u.enable_input(InpSel.ZERO, _L_Z + 1)
    u.require_inp0 = 0
    u.repeat_count = 2
    u.trigger = (Trigger.COUNT, Trigger.NONE, Trigger.NONE)
    u.next_uop = (1, 0, 0)
    dp = u.datapath_config
    for k in range(5):
        dp[k].pass_through_delay(_L_Z)
    for k in range(1, 6):
        b = dp[k]
        b.op = AluOp.BYPASS
        b.alu_src0 = _PD + _L_Z
        b.alu_src1 = b.alu_src0
        b.alu_out_enable = ENABLE
        b.alu_out_a_enable = ENABLE
    return u


def _steady_base(next_idx: int, two_x: bool) -> UopConfig:
    u = UopConfig()
    u.enable_input(InpSel.SRC_0, _L_X + 1)
    u.enable_input(InpSel.CONST_0, _L_W + 1)
    u.enable_input(InpSel.SRC_1, _L_D1 + 1)
    if two_x:
        u.enable_input(InpSel.SRC_0_HI, _L_XH + 1)
        u.enable_input(InpSel.SRC_1_HI, _L_D2 + 1)
    u.require_inp0 = 1
    u.require_inp1 = 1
    u.repeat_count = 1
    u.trigger = (Trigger.SRC_TENSOR_DONE, Trigger.COUNT, Trigger.NONE)
    u.next_uop = (0, next_idx, 0)
    return u


def _chain_alu(u: UopConfig, from_blk: int):
    for k in range(from_blk, 8):
        u.datapath_config[k].pass_through_alu()
    u.enable_output(OutSel.ALU_OUT, OutPath.WR0_LO)


def _lscan_1x(mul: int, state: int, next_idx: int) -> UopConfig:
    u = _steady_base(next_idx, two_x=False)
    dp = u.datapath_config
    for k in range(mul):
        dp[k].pass_through_delay(_L_X, _L_W)
    dp[mul].enable_alu(AluOp.MULTIPLY, _PD + _L_W, AluInp.NEXT_ALU_OUT_A)
    dp[mul].pass_through_delay(_L_X)
    add = dp[state].enable_alu(AluOp.ADD, _PA, _PD + _L_X)
    add.alu_out_a_enable = ENABLE
    _chain_alu(u, state + 1)
    return u


def _lscan_2x(lo_mul: int, next_idx: int) -> UopConfig:
    u = _steady_base(next_idx, two_x=True)
    dp = u.datapath_config
    m0, a0 = lo_mul, lo_mul + 1
    m1, a1 = lo_mul + 2, lo_mul + 3
    for k in range(m0):
        dp[k].pass_through_delay(_L_X, _L_W, _L_XH)
    dp[m0].enable_alu(AluOp.MULTIPLY, _PD + _L_W, AluInp.NEXT_ALU_OUT_A)
    dp[m0].pass_through_delay(_L_X, _L_W, _L_XH)
    addlo = dp[a0].enable_alu(AluOp.ADD, _PA, _PD + _L_X)
    addlo.alu_out_a_enable = ENABLE
    addlo.pass_through_delay(_L_W, _L_XH)
    dp[m1].enable_alu(AluOp.MULTIPLY, _PD + _L_W, AluInp.NEXT_ALU_OUT_A)
    dp[m1].enable_delay_from_src(DelayInp.PREV_ALU_OUT, _L_HL)
    dp[m1].pass_through_delay(_L_XH)
    addhi = dp[a1].enable_alu(AluOp.ADD, _PA, _PD + _L_XH)
    addhi.alu_out_a_enable = ENABLE
    addhi.pass_through_delay(_L_HL)
    for k in range(a1 + 1, 8):
        dp[k].pass_through_alu()
        dp[k].pass_through_delay(_L_HL)
    u.enable_output(OutSel(int(OutSel.DELAY_0) + _L_HL), OutPath.WR0_LO)
    u.enable_output(OutSel.ALU_OUT, OutPath.WR0_HI)
    return u


def _qrec_1x(mul: int, next_idx: int) -> UopConfig:
    u = _steady_base(next_idx, two_x=False)
    dp = u.datapath_config
    for k in range(mul):
        dp[k].pass_through_delay(_L_X, _L_W)
    dp[mul].enable_alu(AluOp.MULTIPLY, _PD + _L_W, AluInp.NEXT_ALU_OUT_A)
    dp[mul].pass_through_delay(_L_X)
    pmax = dp[mul + 1].enable_alu(AluOp.MAX, _PA, _PD + _L_X)
    pmax.alu_out_a_enable = ENABLE
    pmax.pass_through_delay(_L_X)
    dp[mul + 2].enable_alu(AluOp.SUBTRACT, _PA, _PD + _L_X)
    _chain_alu(u, mul + 3)
    if mul + 1 == 6:
        u.accum_enabled = 1
    return u


def _qrec_2x(lo_mul: int, next_idx: int) -> UopConfig:
    u = _steady_base(next_idx, two_x=True)
    dp = u.datapath_config
    m = lo_mul
    for k in range(m):
        dp[k].pass_through_delay(_L_X, _L_W, _L_XH)
    dp[m].enable_alu(AluOp.MULTIPLY, _PD + _L_W, AluInp.NEXT_ALU_OUT_A)
    dp[m].pass_through_delay(_L_X, _L_W, _L_XH)
    pmax = dp[m + 1].enable_alu(AluOp.MAX, _PA, _PD + _L_X)
    pmax.alu_out_a_enable = ENABLE
    pmax.pass_through_delay(_L_X, _L_W, _L_XH)
    dp[m + 2].enable_alu(AluOp.SUBTRACT, _PA, _PD + _L_X)
    dp[m + 2].pass_through_delay(_L_W, _L_XH)
    dp[m + 3].enable_alu(AluOp.MULTIPLY, _PD + _L_W, AluInp.NEXT_ALU_OUT_A)
    dp[m + 3].enable_delay_from_src(DelayInp.PREV_ALU_OUT, _L_HL)
    dp[m + 3].pass_through_delay(_L_XH)
    pmaxh = dp[m + 4].enable_alu(AluOp.MAX, _PA, _PD + _L_XH)
    pmaxh.alu_out_a_enable = ENABLE
    pmaxh.pass_through_delay(_L_XH, _L_HL)
    dp[m + 5].enable_alu(AluOp.SUBTRACT, _PA, _PD + _L_XH)
    dp[m + 5].pass_through_delay(_L_HL)
    for k in range(m + 6, 8):
        dp[k].pass_through_alu()
        dp[k].pass_through_delay(_L_HL)
    u.enable_output(OutSel(int(OutSel.DELAY_0) + _L_HL), OutPath.WR0_LO)
    u.enable_output(OutSel.ALU_OUT, OutPath.WR0_HI)
    return u


def _lscan4_uops():
    # state flops: A@1, B@3, C@2, D@4
    return [
        _seed(),
        _lscan_1x(0, 1, next_idx=2),
        _lscan_1x(2, 3, next_idx=3),
        _lscan_1x(1, 2, next_idx=4),
        _lscan_1x(3, 4, next_idx=1),
    ]


def _lscan4_uops_2x():
    return [_seed(), _lscan_2x(0, next_idx=2), _lscan_2x(1, next_idx=1),
            UopConfig(), UopConfig()]


def _qrec4_uops():
    # state flops: A@1, B@4, C@2, D@5
    return [
        _seed(),
        _qrec_1x(0, next_idx=2),
        _qrec_1x(3, next_idx=3),
        _qrec_1x(1, next_idx=4),
        _qrec_1x(4, next_idx=1),
    ]


def _qrec4_uops_2x():
    return [_seed(), _qrec_2x(0, next_idx=2), _qrec_2x(1, next_idx=1),
            UopConfig(), UopConfig()]


def _ref_lscan4(in0, in1, c0, c1, c2):
    x = np.asarray(in0, np.float32).reshape(in0.shape[0], -1, 4)
    w = np.asarray(c0, np.float32).reshape(-1)
    out = np.empty_like(x)
    for s in range(4):
        acc = np.zeros_like(w)
        for t in range(x.shape[1]):
            acc = w * acc + x[:, t, s]
            out[:, t, s] = acc
    return out.reshape(in0.shape)


def _ref_qrec4(in0, in1, c0, c1, c2):
    el = np.asarray(in0, np.float32).reshape(in0.shape[0], -1, 4)
    w = np.asarray(c0, np.float32).reshape(-1)
    out = np.empty_like(el)
    for s in range(4):
        P_ = np.zeros_like(w)
        for t in range(el.shape[1]):
            v = w * P_
            P_ = np.maximum(el[:, t, s], v)
            out[:, t, s] = P_ - el[:, t, s]
    return out.reshape(in0.shape)


class _HandOp:
    """Duck-typed DveOp with hand-built uop programs (bypasses lower())."""

    def __init__(self, name, spec, uops, uops_2x, perf_max):
        self.name = name
        self.spec = spec
        self.subdim = False
        self._uops = uops
        self._uops_2x = uops_2x
        self._perf_max = perf_max
        self._cache = {}

    def compile(self, ver):
        assert ver == "v3", f"hand-built uops are v3-only, got {ver}"
        if ver not in self._cache:
            s = DveOpSpec(
                name=self.name,
                opcode=dve_ops.get_dve_sub_opcode(self.name),
                uops=self._uops,
                uops_2x=self._uops_2x,
                perf_max=self._perf_max,
                rd1_en=True,
            )
            s.validate(ver)
            self._cache[ver] = s
        return self._cache[ver]


def _register():
    import sys, types

    modname = f"ant_irnn4_ops_{_REV}"
    mod = sys.modules.get(modname)
    if mod is not None:
        return mod.LSCAN4, mod.QREC4
    spec1 = Spec(body=sp_relu(Src0 * C0), reference=_ref_lscan4)
    spec2 = Spec(body=sp_relu(Src0 * C0), reference=_ref_qrec4)
    ls = _HandOp(f"ANT_LSCAN4_{_REV}", spec1, _lscan4_uops(),
                 _lscan4_uops_2x(), perf_max=1)
    qr = _HandOp(f"ANT_QREC4_{_REV}", spec2, _qrec4_uops(),
                 _qrec4_uops_2x(), perf_max=1)
    base = max(dve_ops._SUB_OPCODE_FOR_NAME.values())
    dve_ops._SUB_OPCODE_FOR_NAME[ls.name] = base + 1
    dve_ops._SUB_OPCODE_FOR_NAME[qr.name] = base + 2
    assert max(dve_ops._SUB_OPCODE_FOR_NAME.values()) < 0x20
    dve_ops.OPS.append(ls)
    dve_ops.OPS.append(qr)
    dve_ops.CUSTOM_DVE_SPECS[ls.name] = ls.spec
    dve_ops.CUSTOM_DVE_SPECS[qr.name] = qr.spec
    mod = types.ModuleType(modname)
    mod.LSCAN4, mod.QREC4 = ls, qr
    sys.modules[modname] = mod
    return ls, qr


LSCAN4, QREC4 = _register()

# --- kernel ---

from contextlib import ExitStack

import concourse.bass as bass
import concourse.tile as tile
from concourse import mybir
from concourse.bass_utils import run_bass_kernel_spmd


dt = mybir.dt
Act = mybir.ActivationFunctionType

B, T, I, H, L = 32, 2048, 256, 512, 4
NCORES = 8
BLOC = B // NCORES          # 4 batches per core = one interleave quad
P = 128
TCH = 512
M4 = H // P
KI = I // P
QT = 4 * T                  # quad-interleaved stream length


def build(include_bias=False):
    nc = bass.Bass("TRN2", target_bir_lowering=False, debug=False,
                   num_devices=NCORES)
    # pair-interleaved input: xT[p, i, 2t+j] = x[2p+j, t, i]
    xT_d = nc.dram_tensor("xT", [2, I, 2 * T], dt.float16, kind="ExternalInput").ap()
    w0_d = nc.dram_tensor("w0tn", [I, H], dt.float16, kind="ExternalInput").ap()
    ws_d = nc.dram_tensor("wstn", [L - 1, H, H], dt.float16, kind="ExternalInput").ap()
    wq_d = nc.dram_tensor("wq", [P, L * M4], dt.float32, kind="ExternalInput").ap()
    bias_d = nc.dram_tensor("biasn", [L, 1, H], dt.float16, kind="ExternalInput").ap()
    # quad-interleaved output: [H, (t b)] -- host de-interleaves
    out_d = nc.dram_tensor("out", [H, QT], dt.float16,
                           kind="ExternalOutput").ap()

    with tile.TileContext(nc) as tc, ExitStack() as ctx:
        wpool = ctx.enter_context(tc.tile_pool(name="weights", bufs=1))
        xpool = ctx.enter_context(tc.tile_pool(name="xin", bufs=1))
        spool = ctx.enter_context(tc.tile_pool(name="stage", bufs=1))
        psum = ctx.enter_context(tc.tile_pool(name="psum", bufs=2, space="PSUM"))

        # ---- persistent weights ----
        wq_dmas, crit_dmas, late_dmas = [], [], []
        wqall = wpool.tile([P, L * M4], dt.float32, tag="wqall")
        wq_dmas.append(nc.gpsimd.dma_start(out=wqall[:], in_=wq_d))
        wq = [[wqall[:, (l * M4 + m):(l * M4 + m) + 1] for m in range(M4)]
              for l in range(L)]
        wt = [[] for _ in range(L)]
        for k in range(KI):
            w = wpool.tile([P, H], dt.float16, tag=f"w0{k}")
            crit_dmas.append(nc.gpsimd.dma_start(
                out=w[:], in_=w0_d[k * P:(k + 1) * P, :]))
            wt[0].append(w)
        # input tiles (pair-interleaved): pair 0 split into 4 chunks so the
        # pipeline can start on chunk 0, pair 1 full-tile afterwards
        xin = [[] for _ in range(2)]
        xin_dmas = [[] for _ in range(2)]
        for p in range(2):
            for k in range(KI):
                xt = xpool.tile([P, 2 * T], dt.float16, tag=f"x{p}{k}")
                xin[p].append(xt)
        for c in range(4):
            cs = slice(c * 1024, (c + 1) * 1024)
            for k in range(KI):
                d = nc.gpsimd.dma_start(out=xin[0][k][:, cs],
                                        in_=xT_d[0, k * P:(k + 1) * P, cs])
                crit_dmas.append(d)
                xin_dmas[0].append((c, d))
        for k in range(KI):
            d = nc.gpsimd.dma_start(out=xin[1][k][:],
                                    in_=xT_d[1, k * P:(k + 1) * P, :])
            late_dmas.append(d)
            xin_dmas[1].append((-1, d))
        ws_dmas = []
        for l in range(1, L):
            for k in range(M4):
                w = wpool.tile([P, H], dt.float16, tag=f"w{l}{k}")
                d = nc.gpsimd.dma_start(
                    out=w[:], in_=ws_d[l - 1, k * P:(k + 1) * P, :])
                ws_dmas.append(d)
                wt[l].append(w)
        bias = None
        if include_bias:
            bias = []
            for l in range(L):
                bt = wpool.tile([1, H], dt.float16, tag=f"b{l}")
                ws_dmas.append(nc.gpsimd.dma_start(out=bt[:],
                                                   in_=bias_d[l, :, :]))
                bias.append(bt)
            ones = wpool.tile([1, TCH], dt.float16, tag="ones")
            nc.gpsimd.memset(ones[:], 1.0)

        # ---- absorber machinery (per-engine pinned chains) ----
        scr_v = wpool.tile([P, 160], dt.float32, tag="scr_v")
        scr_a = wpool.tile([P, 160], dt.float32, tag="scr_a")
        state = {"V": [None, 0], "A": [None, 0], "PE": [None]}

        def absorb(eng, dep=None):
            if eng == "V":
                prev, k = state[eng]
                i = nc.vector.tensor_copy(scr_v[:, k:k + 1], wq[0][0][:])
            elif eng == "A":
                prev, k = state[eng]
                i = nc.scalar.activation(scr_a[:, k:k + 1], wq[0][0][:],
                                         Act.Copy)
            else:
                prev = state[eng][0]
                i = nc.tensor.ldweights(weights=wt[0][0][:, 0:P])
            if prev is not None:
                bass._add_dep_helper(i.ins, prev.ins, sync=False, reason="chain")
            if dep is not None:
                bass._add_dep_helper(i.ins, dep.ins, sync=True, reason="absorb")
            if eng == "PE":
                state[eng] = [i]
            else:
                state[eng] = [i, (state[eng][1] + 1) % 160]
            return i

        def pin(real, eng):
            prev = state[eng][0]
            if prev is not None:
                bass._add_dep_helper(real.ins, prev.ins, sync=False, reason="pin")
            state[eng][0] = real
            return real

        # warm-up: junk compute while input DMAs land (p-state ramp)
        jw = wpool.tile([P, P], dt.float16, tag="jw")
        jx = wpool.tile([P, 2048], dt.float16, tag="jx")
        jo = wpool.tile([P, 2048], dt.float16, tag="jo")
        jq = wpool.tile([P, 1], dt.float32, tag="jq")
        nc.vector.memset(jw[:, 0:1], 0.125)
        nc.vector.memset(jx[:, 0:1], 0.125)
        nc.vector.memset(jq[:], 0.125)
        for s in range(2):
            jp = psum.tile([P, T], dt.float32, tag="xp")
            for r in range(12):
                pin(nc.tensor.matmul(jp[:, 0:TCH], lhsT=jw[:],
                                     rhs=jx[:, 0:TCH],
                                     start=True, stop=True), "PE")
        for r in range(3):
            ji = nc.vector._custom_dve(LSCAN4, out=jo[:], in0=jx[:],
                                       in1=jx[:], s0=jq[:])
            ji.ins.perf_max = 1
            pin(ji, "V")

        # engine init
        for eng in ("V", "A"):
            absorb(eng)
            for d in wq_dmas:
                absorb(eng, d)
        for k in range(KI):
            pin(nc.tensor.ldweights(weights=wt[0][k][:, 0:P]), "PE")
        if include_bias:
            for l in range(L):
                pin(nc.tensor.ldweights(weights=bias[l][:, 0:P]), "PE")
            pin(nc.tensor.ldweights(weights=ones[:, 0:P]), "PE")

        # ---- main loop ----
        NXSB, NHP = 2, 7
        hp_ring = []        # (l, m) keys
        hp_by_key = {}
        hp_readers = {}     # key -> last matmul reading it
        xsb_readers = [None] * NXSB   # ring idx -> last QREC4 reader
        psum_readers = [None, None]   # psum slot -> ACT copy that read it
        xsb_i = 0
        psum_i = 0
        hp_i = 0
        last_qrec = None
        out_tiles = []  # layer-3 output tiles; fake end-of-trace readers keep
                        # them live so the allocator never hands a DMA-read
                        # buffer to a later tile (a DMA-completion release dep
                        # would exceed walrus's 1-wait budget)

        for l in range(L):
            kprev = KI if l == 0 else M4
            if l > 0:
                for k in range(M4):
                    pin(nc.tensor.ldweights(weights=wt[l][k][:, 0:P]), "PE")
                # PE absorbs the final QREC4 of layer l-1 (implies all of
                # layer l-1's h tiles via V-queue order)
                absorb("PE", last_qrec)
            for m in range(M4):
                xsb = spool.tile([P, QT], dt.float16, tag="xsb", bufs=NXSB)
                xsb_slot = xsb_i % NXSB
                xsb_i += 1
                last_cp = None
                # one matmul computes a batch PAIR's chunk: moving columns
                # are pair-interleaved, so PSUM holds interleaved xp pairs
                # and ACT writes 2-contiguous/skip-2 into the quad tile
                for p in range(2):
                    if l == 0 and m == 0 and p == 1:
                        for c, d in xin_dmas[1]:
                            absorb("PE", d)
                    for h in range(2):
                        xp = psum.tile([P, T], dt.float32, tag="xp")
                        slot = psum_i % 2
                        psum_i += 1
                        old_rd = psum_readers[slot]
                        if old_rd is not None:
                            absorb("PE", old_rd)
                        last_mm = None
                        for n in range(4):  # 256-timestep chunks
                            ns = slice(n * TCH, (n + 1) * TCH)
                            if l == 0 and p == 0 and m == 0:
                                c_need = h * 2 + n // 2
                                for c, d in xin_dmas[0]:
                                    if c == c_need:
                                        absorb("PE", d)
                            for k in range(kprev):
                                if l == 0:
                                    rhs = xin[p][k][:, h * 2048 + n * TCH:
                                                    h * 2048 + (n + 1) * TCH]
                                else:
                                    t0 = h * 1024 + n * 256
                                    rhs = hp_by_key[(l - 1, k)][:].rearrange(
                                        "p (t s) -> p t s", s=4)[
                                        :, t0:t0 + 256, 2 * p:2 * p + 2]
                                last_mm = nc.tensor.matmul(
                                    xp[:, ns],
                                    lhsT=wt[l][k][:, m * P:(m + 1) * P],
                                    rhs=rhs, start=(k == 0),
                                    stop=(k == kprev - 1 and not include_bias))
                                pin(last_mm, "PE")
                                if l > 0:
                                    hp_readers[(l - 1, k)] = last_mm
                            if include_bias:
                                last_mm = pin(nc.tensor.matmul(
                                    xp[:, ns],
                                    lhsT=bias[l][:, m * P:(m + 1) * P],
                                    rhs=ones[:, :], start=False, stop=True),
                                    "PE")
                        # ACT: PSUM pair-interleaved fp32 -> xsb quad (fp16)
                        old_x = xsb_readers[xsb_slot]
                        if p == 0 and h == 0 and old_x is not None:
                            absorb("A", old_x)
                        absorb("A", last_mm)
                        cp = pin(nc.scalar.activation(
                            xsb[:].rearrange("p (t s) -> p t s", s=4)[
                                :, h * 1024:(h + 1) * 1024, 2 * p:2 * p + 2],
                            xp[:].rearrange("p (t s) -> p t s", s=2),
                            Act.Copy), "A")
                        psum_readers[slot] = cp
                        last_cp = cp
                # DVE: LSCAN4 in-place on xsb, then QREC4 -> hp tile
                absorb("V", last_cp)
                ls = nc.vector._custom_dve(LSCAN4, out=xsb[:], in0=xsb[:],
                                           in1=xsb[:], s0=wq[l][m][:])
                ls.ins.perf_max = 1
                pin(ls, "V")
                hpt = spool.tile([P, QT], dt.float16, tag="hp", bufs=NHP)
                if len(hp_ring) >= NHP:
                    old_key = hp_ring[hp_i % NHP]
                    rd = hp_readers.pop(old_key, None)
                    if rd is not None:
                        absorb("V", rd)
                if len(hp_ring) < NHP:
                    hp_ring.append((l, m))
                else:
                    hp_ring[hp_i % NHP] = (l, m)
                hp_i += 1
                hp_by_key[(l, m)] = hpt
                qr = nc.vector._custom_dve(QREC4, out=hpt[:], in0=xsb[:],
                                           in1=xsb[:], s0=wq[l][m][:])
                qr.ins.perf_max = 1
                pin(qr, "V")
                xsb_readers[xsb_slot] = qr
                last_qrec = qr
                if l == L - 1:
                    # 2 chunks per tile: 8 sync-queue DMAs total (the 9th
                    # would pick up a flow-control wait and trip walrus's
                    # 1-wait budget)
                    for c in range(2):
                        cs = slice(c * (QT // 2), (c + 1) * (QT // 2))
                        nc.sync.dma_start(
                            out=out_d[m * P:(m + 1) * P, cs],
                            in_=hpt[:, cs])
                    out_tiles.append(hpt)

        # fake readers: pin the DMA'd tiles live until end-of-trace
        for ht in out_tiles:
            pin(nc.vector.tensor_copy(scr_v[:, state["V"][1]:state["V"][1] + 1],
                                      ht[:, 0:1]), "V")
            state["V"][1] = (state["V"][1] + 1) % 160

        # ---- tail pre-drains ----
        tail_deps = [i for i in nc.inst_map.values()
                     if type(i).__name__ == "InstDMACopy"]
        snap = list(nc.inst_map.values())
        for eng in ("DVE", "Activation", "PE"):
            last_e = [i for i in snap
                      if str(getattr(i, "engine", "")).endswith(eng)]
            if last_e:
                tail_deps.append(last_e[-1])
        for depi in tail_deps:
            dr = nc.sync.drain(fusable=False)
            bass._add_dep_helper(dr.ins, depi, sync=True,
                                 reason="tail pre-drain absorber")
    assert mybir.codegen_inst_isa_subclasses(nc)
    _assert_wait_budget(nc)
    return nc


_MULTI_WAIT_OK = {"InstDrain",
                  "InstEventSemaphore", "InstUnconditionalBranch",
                  "InstRegisterMove", "InstISA", "InstTensorLoad",
                  "InstTensorSave"}


def _assert_wait_budget(nc):
    bad = []
    for name, inst in nc.inst_map.items():
        ty = type(inst).__name__
        w = inst.sync_info.on_wait if inst.sync_info else []
        if ty == "InstCustomDveAnt":
            fw = [x for x in w if not x.ant_name.startswith("DVE")]
            if fw:
                bad.append((name, ty, [f"{x.ant_name}>={x.wait_value}"
                                       for x in fw]))
            continue
        if ty in _MULTI_WAIT_OK:
            continue
        if len(w) > 1:
            bad.append((name, ty,
                        [f"{x.ant_name}>={x.wait_value}" for x in w]))
    if bad:
        raise RuntimeError(
            f"{len(bad)} instructions exceed the sync-wait budget, "
            f"first few: {bad[:6]}")


def _prep_core_inputs(Input, W0, Ws, bs, whs, core):
    bsl = slice(core * BLOC, (core + 1) * BLOC)
    xb = Input[bsl]                      # [4, T, I]
    # pair-interleave: xT[p, i, 2t+j] = xb[2p+j, t, i]
    xT = np.ascontiguousarray(
        xb.reshape(2, 2, T, I).transpose(0, 3, 2, 1).reshape(2, I, 2 * T))
    return {
        "xT": xT.astype(np.float16),
        "w0tn": np.ascontiguousarray(-W0.T).astype(np.float16),
        "wstn": np.ascontiguousarray(-Ws.transpose(0, 2, 1)).astype(np.float16),
        "wq": np.ascontiguousarray(
            whs.astype(np.float32).reshape(L, M4, P).transpose(2, 0, 1)
            .reshape(P, L * M4)),
        "biasn": np.ascontiguousarray(-bs[:, None, :]).astype(np.float16),
    }


def kernel(Input, W0, Ws, bs, whs):
    include_bias = bool(np.any(bs != 0))
    nc = build(include_bias=include_bias)
    in_maps = [_prep_core_inputs(Input, W0, Ws, bs, whs, r)
               for r in range(NCORES)]
    res = run_bass_kernel_spmd(nc, in_maps, core_ids=list(range(NCORES)))
    parts = []
    for r in range(NCORES):
        o = res.results[r]["out"]  # [H, 4T] quad-interleaved
        o = o.reshape(H, T, BLOC).transpose(2, 0, 1)  # [BLOC, H, T]
        parts.append(o)
    full = np.concatenate(parts, axis=0)  # [B, H, T]
    return np.ascontiguousarray(full.transpose(0, 2, 1)).astype(np.float32)


# revision 22
# speedup vs baseline: 1.8372x; 1.0413x over previous
"""Trainium2 Bass kernel for a 4-layer IndRNN (B=32, T=2048, I=256, H=512).

v3: 4-stream interleaved custom DVE ops (ANT_LSCAN4 / ANT_QREC4) with
2X_1PORT uop programs run the whole recurrence at 2 fp16 elem/cycle.
All 4 batches of a core are element-interleaved (a_t,b_t,c_t,d_t,...) in
one [128, 4T] fp16 stream per (layer, m-tile); one LSCAN4 + one QREC4
per group replaces the baseline's four pair ops at half the DVE time.

Math: per layer, with PSUM holding -xp (weights negated on host):
    l_t = w*l_{t-1} + (-xp_t)                     (LSCAN4; in-place)
    v_t = w*P_{t-1}; P_t = max(l_t, v_t); h_t = P_t - l_t   (QREC4)
which equals h_t = relu(xp_t + w*h_{t-1}), the IndRNN layer
(P - l == max(v - l, 0) saves an ALU stage -> 3 ops/elem, so two
elements fit the 8-stage datapath in 2x mode).

2x mode notes: rd1_en=1 with in1 := in0 forces the handler's TwoSrc perf
enable so only 2X_1PORT is reachable (2X_2PORT/4X would feed the uops a
port layout they can't drain -> engine hang); the uops consume SRC_1 into
dummy lanes. perf_max=1 is set on each instruction (byte-36[7:6]).

Sharding: data-parallel over batch, 4 batches (= 1 quad) per core.
"""

import numpy as np

from concourse import dve_ops
from concourse.dve_spec import Spec, Src0, C0, relu as sp_relu
from concourse.dve_uop import (
    AluInp,
    AluOp,
    DelayInp,
    DveOpSpec,
    ENABLE,
    InpSel,
    OutPath,
    OutSel,
    Trigger,
    UopConfig,
)

_REV = "r3"

# lanes: X=SRC_0(LO), W=CONST_0, XH=SRC_0_HI, HL=LO-result carry (2x),
# D1/D2 = dummy sinks for SRC_1/SRC_1_HI (consumed, never read)
_L_X, _L_W, _L_XH, _L_HL, _L_Z = 0, 1, 2, 3, 4
_L_D1, _L_D2 = 4, 5

_PD = AluInp.PREV_DELAY_0  # + lane id
_PA = AluInp.PREV_ALU_OUT


def _seed() -> UopConfig:
    """Zero the a-flops at stages 1..5 (superset of both ops' state flops)."""
    u = UopConfig()
    u.enable_input(InpSel.ZERO, _L_Z + 1)
    u.require_inp0 = 0
    u.repeat_count = 2
    u.trigger = (Trigger.COUNT, Trigger.NONE, Trigger.NONE)
    u.next_uop = (1, 0, 0)
    dp = u.datapath_config
    for k in range(5):
        dp[k].pass_through_delay(_L_Z)
    for k in range(1, 6):
        b = dp[k]
        b.op = AluOp.BYPASS
        b.alu_src0 = _PD + _L_Z
        b.alu_src1 = b.alu_src0
        b.alu_out_enable = ENABLE
        b.alu_out_a_enable = ENABLE
    return u


def _steady_base(next_idx: int, two_x: bool) -> UopConfig:
    u = UopConfig()
    u.enable_input(InpSel.SRC_0, _L_X + 1)
    u.enable_input(InpSel.CONST_0, _L_W + 1)
    u.enable_input(InpSel.SRC_1, _L_D1 + 1)
    if two_x:
        u.enable_input(InpSel.SRC_0_HI, _L_XH + 1)
        u.enable_input(InpSel.SRC_1_HI, _L_D2 + 1)
    u.require_inp0 = 1
    u.require_inp1 = 1
    u.repeat_count = 1
    u.trigger = (Trigger.SRC_TENSOR_DONE, Trigger.COUNT, Trigger.NONE)
    u.next_uop = (0, next_idx, 0)
    return u


def _chain_alu(u: UopConfig, from_blk: int):
    for k in range(from_blk, 8):
        u.datapath_config[k].pass_through_alu()
    u.enable_output(OutSel.ALU_OUT, OutPath.WR0_LO)


def _lscan_1x(mul: int, state: int, next_idx: int) -> UopConfig:
    u = _steady_base(next_idx, two_x=False)
    dp = u.datapath_config
    for k in range(mul):
        dp[k].pass_through_delay(_L_X, _L_W)
    dp[mul].enable_alu(AluOp.MULTIPLY, _PD + _L_W, AluInp.NEXT_ALU_OUT_A)
    dp[mul].pass_through_delay(_L_X)
    add = dp[state].enable_alu(AluOp.ADD, _PA, _PD + _L_X)
    add.alu_out_a_enable = ENABLE
    _chain_alu(u, state + 1)
    return u


def _lscan_2x(lo_mul: int, next_idx: int) -> UopConfig:
    u = _steady_base(next_idx, two_x=True)
    dp = u.datapath_config
    m0, a0 = lo_mul, lo_mul + 1
    m1, a1 = lo_mul + 2, lo_mul + 3
    for k in range(m0):
        dp[k].pass_through_delay(_L_X, _L_W, _L_XH)
    dp[m0].enable_alu(AluOp.MULTIPLY, _PD + _L_W, AluInp.NEXT_ALU_OUT_A)
    dp[m0].pass_through_delay(_L_X, _L_W, _L_XH)
    addlo = dp[a0].enable_alu(AluOp.ADD, _PA, _PD + _L_X)
    addlo.alu_out_a_enable = ENABLE
    addlo.pass_through_delay(_L_W, _L_XH)
    dp[m1].enable_alu(AluOp.MULTIPLY, _PD + _L_W, AluInp.NEXT_ALU_OUT_A)
    dp[m1].enable_delay_from_src(DelayInp.PREV_ALU_OUT, _L_HL)
    dp[m1].pass_through_delay(_L_XH)
    addhi = dp[a1].enable_alu(AluOp.ADD, _PA, _PD + _L_XH)
    addhi.alu_out_a_enable = ENABLE
    addhi.pass_through_delay(_L_HL)
    for k in range(a1 + 1, 8):
        dp[k].pass_through_alu()
        dp[k].pass_through_delay(_L_HL)
    u.enable_output(OutSel(int(OutSel.DELAY_0) + _L_HL), OutPath.WR0_LO)
    u.enable_output(OutSel.ALU_OUT, OutPath.WR0_HI)
    return u


def _qrec_1x(mul: int, next_idx: int) -> UopConfig:
    u = _steady_base(next_idx, two_x=False)
    dp = u.datapath_config
    for k in range(mul):
        dp[k].pass_through_delay(_L_X, _L_W)
    dp[mul].enable_alu(AluOp.MULTIPLY, _PD + _L_W, AluInp.NEXT_ALU_OUT_A)
    dp[mul].pass_through_delay(_L_X)
    pmax = dp[mul + 1].enable_alu(AluOp.MAX, _PA, _PD + _L_X)
    pmax.alu_out_a_enable = ENABLE
    pmax.pass_through_delay(_L_X)
    dp[mul + 2].enable_alu(AluOp.SUBTRACT, _PA, _PD + _L_X)
    _chain_alu(u, mul + 3)
    if mul + 1 == 6:
        u.accum_enabled = 1
    return u


def _qrec_2x(lo_mul: int, next_idx: int) -> UopConfig:
    u = _steady_base(next_idx, two_x=True)
    dp = u.datapath_config
    m = lo_mul
    for k in range(m):
        dp[k].pass_through_delay(_L_X, _L_W, _L_XH)
    dp[m].enable_alu(AluOp.MULTIPLY, _PD + _L_W, AluInp.NEXT_ALU_OUT_A)
    dp[m].pass_through_delay(_L_X, _L_W, _L_XH)
    pmax = dp[m + 1].enable_alu(AluOp.MAX, _PA, _PD + _L_X)
    pmax.alu_out_a_enable = ENABLE
    pmax.pass_through_delay(_L_X, _L_W, _L_XH)
    dp[m + 2].enable_alu(AluOp.SUBTRACT, _PA, _PD + _L_X)
    dp[m + 2].pass_through_delay(_L_W, _L_XH)
    dp[m + 3].enable_alu(AluOp.MULTIPLY, _PD + _L_W, AluInp.NEXT_ALU_OUT_A)
    dp[m + 3].enable_delay_from_src(DelayInp.PREV_ALU_OUT, _L_HL)
    dp[m + 3].pass_through_delay(_L_XH)
    pmaxh = dp[m + 4].enable_alu(AluOp.MAX, _PA, _PD + _L_XH)
    pmaxh.alu_out_a_enable = ENABLE
    pmaxh.pass_through_delay(_L_XH, _L_HL)
    dp[m + 5].enable_alu(AluOp.SUBTRACT, _PA, _PD + _L_XH)
    dp[m + 5].pass_through_delay(_L_HL)
    for k in range(m + 6, 8):
        dp[k].pass_through_alu()
        dp[k].pass_through_delay(_L_HL)
    u.enable_output(OutSel(int(OutSel.DELAY_0) + _L_HL), OutPath.WR0_LO)
    u.enable_output(OutSel.ALU_OUT, OutPath.WR0_HI)
    return u


def _lscan4_uops():
    # state flops: A@1, B@3, C@2, D@4
    return [
        _seed(),
        _lscan_1x(0, 1, next_idx=2),
        _lscan_1x(2, 3, next_idx=3),
        _lscan_1x(1, 2, next_idx=4),
        _lscan_1x(3, 4, next_idx=1),
    ]


def _lscan4_uops_2x():
    return [_seed(), _lscan_2x(0, next_idx=2), _lscan_2x(1, next_idx=1),
            UopConfig(), UopConfig()]


def _qrec4_uops():
    # state flops: A@1, B@4, C@2, D@5
    return [
        _seed(),
        _qrec_1x(0, next_idx=2),
        _qrec_1x(3, next_idx=3),
        _qrec_1x(1, next_idx=4),
        _qrec_1x(4, next_idx=1),
    ]


def _qrec4_uops_2x():
    return [_seed(), _qrec_2x(0, next_idx=2), _qrec_2x(1, next_idx=1),
            UopConfig(), UopConfig()]


def _ref_lscan4(in0, in1, c0, c1, c2):
    x = np.asarray(in0, np.float32).reshape(in0.shape[0], -1, 4)
    w = np.asarray(c0, np.float32).reshape(-1)
    out = np.empty_like(x)
    for s in range(4):
        acc = np.zeros_like(w)
        for t in range(x.shape[1]):
            acc = w * acc + x[:, t, s]
            out[:, t, s] = acc
    return out.reshape(in0.shape)


def _ref_qrec4(in0, in1, c0, c1, c2):
    el = np.asarray(in0, np.float32).reshape(in0.shape[0], -1, 4)
    w = np.asarray(c0, np.float32).reshape(-1)
    out = np.empty_like(el)
    for s in range(4):
        P_ = np.zeros_like(w)
        for t in range(el.shape[1]):
            v = w * P_
            P_ = np.maximum(el[:, t, s], v)
            out[:, t, s] = P_ - el[:, t, s]
    return out.reshape(in0.shape)


class _HandOp:
    """Duck-typed DveOp with hand-built uop programs (bypasses lower())."""

    def __init__(self, name, spec, uops, uops_2x, perf_max):
        self.name = name
        self.spec = spec
        self.subdim = False
        self._uops = uops
        self._uops_2x = uops_2x
        self._perf_max = perf_max
        self._cache = {}

    def compile(self, ver):
        assert ver == "v3", f"hand-built uops are v3-only, got {ver}"
        if ver not in self._cache:
            s = DveOpSpec(
                name=self.name,
                opcode=dve_ops.get_dve_sub_opcode(self.name),
                uops=self._uops,
                uops_2x=self._uops_2x,
                perf_max=self._perf_max,
                rd1_en=True,
            )
            s.validate(ver)
            self._cache[ver] = s
        return self._cache[ver]


def _register():
    import sys, types

    modname = f"ant_irnn4_ops_{_REV}"
    mod = sys.modules.get(modname)
    if mod is not None:
        return mod.LSCAN4, mod.QREC4
    spec1 = Spec(body=sp_relu(Src0 * C0), reference=_ref_lscan4)
    spec2 = Spec(body=sp_relu(Src0 * C0), reference=_ref_qrec4)
    ls = _HandOp(f"ANT_LSCAN4_{_REV}", spec1, _lscan4_uops(),
                 _lscan4_uops_2x(), perf_max=1)
    qr = _HandOp(f"ANT_QREC4_{_REV}", spec2, _qrec4_uops(),
                 _qrec4_uops_2x(), perf_max=1)
    base = max(dve_ops._SUB_OPCODE_FOR_NAME.values())
    dve_ops._SUB_OPCODE_FOR_NAME[ls.name] = base + 1
    dve_ops._SUB_OPCODE_FOR_NAME[qr.name] = base + 2
    assert max(dve_ops._SUB_OPCODE_FOR_NAME.values()) < 0x20
    dve_ops.OPS.append(ls)
    dve_ops.OPS.append(qr)
    dve_ops.CUSTOM_DVE_SPECS[ls.name] = ls.spec
    dve_ops.CUSTOM_DVE_SPECS[qr.name] = qr.spec
    mod = types.ModuleType(modname)
    mod.LSCAN4, mod.QREC4 = ls, qr
    sys.modules[modname] = mod
    return ls, qr


LSCAN4, QREC4 = _register()

# --- kernel ---

from contextlib import ExitStack

import concourse.bass as bass
import concourse.tile as tile
from concourse import mybir
from concourse.bass_utils import run_bass_kernel_spmd


dt = mybir.dt
Act = mybir.ActivationFunctionType

B, T, I, H, L = 32, 2048, 256, 512, 4
NCORES = 8
BLOC = B // NCORES          # 4 batches per core = one interleave quad
P = 128
TCH = 512
M4 = H // P
KI = I // P
QT = 4 * T                  # quad-interleaved stream length


def build(include_bias=False):
    nc = bass.Bass("TRN2", target_bir_lowering=False, debug=False,
                   num_devices=NCORES)
    # pair-interleaved input: xT[p, i, 2t+j] = x[2p+j, t, i]
    xT_d = nc.dram_tensor("xT", [2, I, 2 * T], dt.float16, kind="ExternalInput").ap()
    w0_d = nc.dram_tensor("w0tn", [I, H], dt.float16, kind="ExternalInput").ap()
    ws_d = nc.dram_tensor("wstn", [L - 1, H, H], dt.float16, kind="ExternalInput").ap()
    wq_d = nc.dram_tensor("wq", [P, L * M4], dt.float32, kind="ExternalInput").ap()
    bias_d = nc.dram_tensor("biasn", [L, 1, H], dt.float16, kind="ExternalInput").ap()
    # quad-interleaved output: [H, (t b)] -- host de-interleaves
    out_d = nc.dram_tensor("out", [H, QT], dt.float16,
                           kind="ExternalOutput").ap()

    with tile.TileContext(nc) as tc, ExitStack() as ctx:
        wpool = ctx.enter_context(tc.tile_pool(name="weights", bufs=1))
        xpool = ctx.enter_context(tc.tile_pool(name="xin", bufs=1))
        spool = ctx.enter_context(tc.tile_pool(name="stage", bufs=1))
        psum = ctx.enter_context(tc.tile_pool(name="psum", bufs=2, space="PSUM"))

        # ---- persistent weights ----
        wq_dmas, crit_dmas, late_dmas = [], [], []
        wqall = wpool.tile([P, L * M4], dt.float32, tag="wqall")
        wq_dmas.append(nc.gpsimd.dma_start(out=wqall[:], in_=wq_d))
        wq = [[wqall[:, (l * M4 + m):(l * M4 + m) + 1] for m in range(M4)]
              for l in range(L)]
        wt = [[] for _ in range(L)]
        for k in range(KI):
            w = wpool.tile([P, H], dt.float16, tag=f"w0{k}")
            crit_dmas.append(nc.gpsimd.dma_start(
                out=w[:], in_=w0_d[k * P:(k + 1) * P, :]))
            wt[0].append(w)
        # input tiles (pair-interleaved): pair 0 split into 4 chunks so the
        # pipeline can start on chunk 0, pair 1 full-tile afterwards
        xin = [[] for _ in range(2)]
        xin_dmas = [[] for _ in range(2)]
        for p in range(2):
            for k in range(KI):
                xt = xpool.tile([P, 2 * T], dt.float16, tag=f"x{p}{k}")
                xin[p].append(xt)
        for c in range(8):
            cs = slice(c * 512, (c + 1) * 512)
            for k in range(KI):
                d = nc.gpsimd.dma_start(out=xin[0][k][:, cs],
                                        in_=xT_d[0, k * P:(k + 1) * P, cs])
                crit_dmas.append(d)
                xin_dmas[0].append((c, d))
        for k in range(KI):
            d = nc.gpsimd.dma_start(out=xin[1][k][:],
                                    in_=xT_d[1, k * P:(k + 1) * P, :])
            late_dmas.append(d)
            xin_dmas[1].append((-1, d))
        ws_dmas = []
        for l in range(1, L):
            for k in range(M4):
                w = wpool.tile([P, H], dt.float16, tag=f"w{l}{k}")
                d = nc.gpsimd.dma_start(
                    out=w[:], in_=ws_d[l - 1, k * P:(k + 1) * P, :])
                ws_dmas.append(d)
                wt[l].append(w)
        bias = None
        if include_bias:
            bias = []
            for l in range(L):
                bt = wpool.tile([1, H], dt.float16, tag=f"b{l}")
                ws_dmas.append(nc.gpsimd.dma_start(out=bt[:],
                                                   in_=bias_d[l, :, :]))
                bias.append(bt)
            ones = wpool.tile([1, TCH], dt.float16, tag="ones")
            nc.gpsimd.memset(ones[:], 1.0)

        # ---- absorber machinery (per-engine pinned chains) ----
        scr_v = wpool.tile([P, 160], dt.float32, tag="scr_v")
        scr_a = wpool.tile([P, 160], dt.float32, tag="scr_a")
        state = {"V": [None, 0], "A": [None, 0], "PE": [None]}

        def absorb(eng, dep=None):
            if eng == "V":
                prev, k = state[eng]
                i = nc.vector.tensor_copy(scr_v[:, k:k + 1], wq[0][0][:])
            elif eng == "A":
                prev, k = state[eng]
                i = nc.scalar.activation(scr_a[:, k:k + 1], wq[0][0][:],
                                         Act.Copy)
            else:
                prev = state[eng][0]
                i = nc.tensor.ldweights(weights=wt[0][0][:, 0:P])
            if prev is not None:
                bass._add_dep_helper(i.ins, prev.ins, sync=False, reason="chain")
            if dep is not None:
                bass._add_dep_helper(i.ins, dep.ins, sync=True, reason="absorb")
            if eng == "PE":
                state[eng] = [i]
            else:
                state[eng] = [i, (state[eng][1] + 1) % 160]
            return i

        def pin(real, eng):
            prev = state[eng][0]
            if prev is not None:
                bass._add_dep_helper(real.ins, prev.ins, sync=False, reason="pin")
            state[eng][0] = real
            return real

        # warm-up: junk compute while input DMAs land (p-state ramp)
        jw = wpool.tile([P, P], dt.float16, tag="jw")
        jx = wpool.tile([P, 2048], dt.float16, tag="jx")
        jo = wpool.tile([P, 2048], dt.float16, tag="jo")
        jq = wpool.tile([P, 1], dt.float32, tag="jq")
        nc.vector.memset(jw[:, 0:1], 0.125)
        nc.vector.memset(jx[:, 0:1], 0.125)
        nc.vector.memset(jq[:], 0.125)
        for s in range(2):
            jp = psum.tile([P, T], dt.float32, tag="xp")
            for r in range(12):
                pin(nc.tensor.matmul(jp[:, 0:TCH], lhsT=jw[:],
                                     rhs=jx[:, 0:TCH],
                                     start=True, stop=True), "PE")
        for r in range(3):
            ji = nc.vector._custom_dve(LSCAN4, out=jo[:], in0=jx[:],
                                       in1=jx[:], s0=jq[:])
            ji.ins.perf_max = 1
            pin(ji, "V")

        # engine init
        for eng in ("V", "A"):
            absorb(eng)
            for d in wq_dmas:
                absorb(eng, d)
        for k in range(KI):
            pin(nc.tensor.ldweights(weights=wt[0][k][:, 0:P]), "PE")
        if include_bias:
            for l in range(L):
                pin(nc.tensor.ldweights(weights=bias[l][:, 0:P]), "PE")
            pin(nc.tensor.ldweights(weights=ones[:, 0:P]), "PE")

        # ---- main loop ----
        NXSB, NHP = 2, 7
        hp_ring = []        # (l, m) keys
        hp_by_key = {}
        hp_readers = {}     # key -> last matmul reading it
        xsb_readers = [None] * NXSB   # ring idx -> last QREC4 reader
        psum_readers = [None, None]   # psum slot -> ACT copy that read it
        xsb_i = 0
        psum_i = 0
        hp_i = 0
        last_qrec = None
        qrec_by = {}
        out_tiles = []  # layer-3 output tiles; fake end-of-trace readers keep
                        # them live so the allocator never hands a DMA-read
                        # buffer to a later tile (a DMA-completion release dep
                        # would exceed walrus's 1-wait budget)

        def mm_rhs(l, k, p, h, n):
            if l == 0:
                return xin[p][k][:, h * 2048 + n * TCH:
                                 h * 2048 + (n + 1) * TCH]
            t0 = h * 1024 + n * 256
            return hp_by_key[(l - 1, k)][:].rearrange(
                "p (t s) -> p t s", s=4)[:, t0:t0 + 256, 2 * p:2 * p + 2]

        def emit_mms(l, m, p, h, xp, ks, stop_k):
            """Emit the k in `ks` matmuls for all 4 chunks of group (p, h).
            stop_k: the k that carries stop (None while pre-issuing)."""
            kprev = KI if l == 0 else M4
            last_mm = None
            for n in range(4):  # 256-timestep chunks
                ns = slice(n * TCH, (n + 1) * TCH)
                if l == 0 and p == 0 and m == 0:
                    c_need = h * 4 + n
                    for c, d in xin_dmas[0]:
                        if c == c_need:
                            absorb("PE", d)
                for k in ks:
                    last_mm = nc.tensor.matmul(
                        xp[:, ns], lhsT=wt[l][k][:, m * P:(m + 1) * P],
                        rhs=mm_rhs(l, k, p, h, n), start=(k == 0),
                        stop=(k == stop_k and not include_bias))
                    pin(last_mm, "PE")
                    if l > 0:
                        hp_readers[(l - 1, k)] = last_mm
                if include_bias and stop_k is not None and stop_k in ks:
                    last_mm = pin(nc.tensor.matmul(
                        xp[:, ns], lhsT=bias[l][:, m * P:(m + 1) * P],
                        rhs=ones[:, :], start=False, stop=True), "PE")
            return last_mm

        for l in range(L):
            kprev = KI if l == 0 else M4
            if l > 0:
                for k in range(M4):
                    pin(nc.tensor.ldweights(weights=wt[l][k][:, 0:P]), "PE")
            for m in range(M4):
                xsb = spool.tile([P, QT], dt.float16, tag="xsb", bufs=NXSB)
                xsb_slot = xsb_i % NXSB
                xsb_i += 1
                last_cp = None
                # one matmul computes a batch PAIR's chunk: moving columns
                # are pair-interleaved, so PSUM holds interleaved xp pairs
                # and ACT writes 2-contiguous/skip-2 into the quad tile.
                # At a layer boundary, pre-issue k<last for the first two
                # groups before waiting on the previous layer's final QREC4
                # so PE stays busy (and HAM stays warm) through the handoff.
                pre = {}
                if l > 0 and m == 0:
                    # hp(l-1, 0..2) finished long ago; absorbing their QREC4s
                    # lets the k<3 pre-issue run without foreign waits
                    absorb("PE", qrec_by[(l - 1, 2)])
                    for p, h in ((0, 0), (0, 1)):
                        xp = psum.tile([P, T], dt.float32, tag="xp")
                        slot = psum_i % 2
                        psum_i += 1
                        old_rd = psum_readers[slot]
                        if old_rd is not None:
                            absorb("PE", old_rd)
                        emit_mms(l, m, p, h, xp, range(kprev - 1), None)
                        pre[(p, h)] = (xp, slot)
                    absorb("PE", last_qrec)
                for p in range(2):
                    if l == 0 and m == 0 and p == 1:
                        for c, d in xin_dmas[1]:
                            absorb("PE", d)
                    for h in range(2):
                        if (p, h) in pre:
                            xp, slot = pre[(p, h)]
                            last_mm = emit_mms(l, m, p, h, xp,
                                               [kprev - 1], kprev - 1)
                        else:
                            xp = psum.tile([P, T], dt.float32, tag="xp")
                            slot = psum_i % 2
                            psum_i += 1
                            old_rd = psum_readers[slot]
                            if old_rd is not None:
                                absorb("PE", old_rd)
                            last_mm = emit_mms(l, m, p, h, xp,
                                               range(kprev), kprev - 1)
                        # ACT: PSUM pair-interleaved fp32 -> xsb quad (fp16)
                        old_x = xsb_readers[xsb_slot]
                        if p == 0 and h == 0 and old_x is not None:
                            absorb("A", old_x)
                        absorb("A", last_mm)
                        cp = pin(nc.scalar.activation(
                            xsb[:].rearrange("p (t s) -> p t s", s=4)[
                                :, h * 1024:(h + 1) * 1024, 2 * p:2 * p + 2],
                            xp[:].rearrange("p (t s) -> p t s", s=2),
                            Act.Copy), "A")
                        psum_readers[slot] = cp
                        last_cp = cp
                # DVE: LSCAN4 in-place on xsb, then QREC4 -> hp tile
                absorb("V", last_cp)
                ls = nc.vector._custom_dve(LSCAN4, out=xsb[:], in0=xsb[:],
                                           in1=xsb[:], s0=wq[l][m][:])
                ls.ins.perf_max = 1
                pin(ls, "V")
                hpt = spool.tile([P, QT], dt.float16, tag="hp", bufs=NHP)
                if len(hp_ring) >= NHP:
                    old_key = hp_ring[hp_i % NHP]
                    rd = hp_readers.pop(old_key, None)
                    if rd is not None:
                        absorb("V", rd)
                if len(hp_ring) < NHP:
                    hp_ring.append((l, m))
                else:
                    hp_ring[hp_i % NHP] = (l, m)
                hp_i += 1
                hp_by_key[(l, m)] = hpt
                qr = nc.vector._custom_dve(QREC4, out=hpt[:], in0=xsb[:],
                                           in1=xsb[:], s0=wq[l][m][:])
                qr.ins.perf_max = 1
                pin(qr, "V")
                xsb_readers[xsb_slot] = qr
                last_qrec = qr
                qrec_by[(l, m)] = qr
                if l == L - 1:
                    # 2 chunks per tile: 8 sync-queue DMAs total (the 9th
                    # would pick up a flow-control wait and trip walrus's
                    # 1-wait budget)
                    for c in range(2):
                        cs = slice(c * (QT // 2), (c + 1) * (QT // 2))
                        nc.sync.dma_start(
                            out=out_d[m * P:(m + 1) * P, cs],
                            in_=hpt[:, cs])
                    out_tiles.append(hpt)

        # fake readers: pin the DMA'd tiles live until end-of-trace
        for ht in out_tiles:
            pin(nc.vector.tensor_copy(scr_v[:, state["V"][1]:state["V"][1] + 1],
                                      ht[:, 0:1]), "V")
            state["V"][1] = (state["V"][1] + 1) % 160

        # ---- tail pre-drains ----
        tail_deps = [i for i in nc.inst_map.values()
                     if type(i).__name__ == "InstDMACopy"]
        snap = list(nc.inst_map.values())
        for eng in ("DVE", "Activation", "PE"):
            last_e = [i for i in snap
                      if str(getattr(i, "engine", "")).endswith(eng)]
            if last_e:
                tail_deps.append(last_e[-1])
        for depi in tail_deps:
            dr = nc.sync.drain(fusable=False)
            bass._add_dep_helper(dr.ins, depi, sync=True,
                                 reason="tail pre-drain absorber")
    assert mybir.codegen_inst_isa_subclasses(nc)
    _assert_wait_budget(nc)
    return nc


_MULTI_WAIT_OK = {"InstDrain",
                  "InstEventSemaphore", "InstUnconditionalBranch",
                  "InstRegisterMove", "InstISA", "InstTensorLoad",
                  "InstTensorSave"}


def _assert_wait_budget(nc):
    bad = []
    for name, inst in nc.inst_map.items():
        ty = type(inst).__name__
        w = inst.sync_info.on_wait if inst.sync_info else []
        if ty == "InstCustomDveAnt":
            fw = [x for x in w if not x.ant_name.startswith("DVE")]
            if fw:
                bad.append((name, ty, [f"{x.ant_name}>={x.wait_value}"
                                       for x in fw]))
            continue
        if ty in _MULTI_WAIT_OK:
            continue
        if len(w) > 1:
            bad.append((name, ty,
                        [f"{x.ant_name}>={x.wait_value}" for x in w]))
    if bad:
        raise RuntimeError(
            f"{len(bad)} instructions exceed the sync-wait budget, "
            f"first few: {bad[:6]}")


def _prep_core_inputs(Input, W0, Ws, bs, whs, core):
    bsl = slice(core * BLOC, (core + 1) * BLOC)
    xb = Input[bsl]                      # [4, T, I]
    # pair-interleave: xT[p, i, 2t+j] = xb[2p+j, t, i]
    xT = np.ascontiguousarray(
        xb.reshape(2, 2, T, I).transpose(0, 3, 2, 1).reshape(2, I, 2 * T))
    return {
        "xT": xT.astype(np.float16),
        "w0tn": np.ascontiguousarray(-W0.T).astype(np.float16),
        "wstn": np.ascontiguousarray(-Ws.transpose(0, 2, 1)).astype(np.float16),
        "wq": np.ascontiguousarray(
            whs.astype(np.float32).reshape(L, M4, P).transpose(2, 0, 1)
            .reshape(P, L * M4)),
        "biasn": np.ascontiguousarray(-bs[:, None, :]).astype(np.float16),
    }


def kernel(Input, W0, Ws, bs, whs):
    include_bias = bool(np.any(bs != 0))
    nc = build(include_bias=include_bias)
    in_maps = [_prep_core_inputs(Input, W0, Ws, bs, whs, r)
               for r in range(NCORES)]
    res = run_bass_kernel_spmd(nc, in_maps, core_ids=list(range(NCORES)))
    parts = []
    for r in range(NCORES):
        o = res.results[r]["out"]  # [H, 4T] quad-interleaved
        o = o.reshape(H, T, BLOC).transpose(2, 0, 1)  # [BLOC, H, T]
        parts.append(o)
    full = np.concatenate(parts, axis=0)  # [B, H, T]
    return np.ascontiguousarray(full.transpose(0, 2, 1)).astype(np.float32)


# revision 27
# speedup vs baseline: 1.8622x; 1.0136x over previous
"""Trainium2 Bass kernel for a 4-layer IndRNN (B=32, T=2048, I=256, H=512).

v3: 4-stream interleaved custom DVE ops (ANT_LSCAN4 / ANT_QREC4) with
2X_1PORT uop programs run the whole recurrence at 2 fp16 elem/cycle.
All 4 batches of a core are element-interleaved (a_t,b_t,c_t,d_t,...) in
one [128, 4T] fp16 stream per (layer, m-tile); one LSCAN4 + one QREC4
per group replaces the baseline's four pair ops at half the DVE time.

Math: per layer, with PSUM holding -xp (weights negated on host):
    l_t = w*l_{t-1} + (-xp_t)                     (LSCAN4; in-place)
    v_t = w*P_{t-1}; P_t = max(l_t, v_t); h_t = P_t - l_t   (QREC4)
which equals h_t = relu(xp_t + w*h_{t-1}), the IndRNN layer
(P - l == max(v - l, 0) saves an ALU stage -> 3 ops/elem, so two
elements fit the 8-stage datapath in 2x mode).

2x mode notes: rd1_en=1 with in1 := in0 forces the handler's TwoSrc perf
enable so only 2X_1PORT is reachable (2X_2PORT/4X would feed the uops a
port layout they can't drain -> engine hang); the uops consume SRC_1 into
dummy lanes. perf_max=1 is set on each instruction (byte-36[7:6]).

Sharding: data-parallel over batch, 4 batches (= 1 quad) per core.
"""

import numpy as np

from concourse import dve_ops
from concourse.dve_spec import Spec, Src0, C0, relu as sp_relu
from concourse.dve_uop import (
    AluInp,
    AluOp,
    DelayInp,
    DveOpSpec,
    ENABLE,
    InpSel,
    OutPath,
    OutSel,
    Trigger,
    UopConfig,
)

_REV = "r3"

# lanes: X=SRC_0(LO), W=CONST_0, XH=SRC_0_HI, HL=LO-result carry (2x),
# D1/D2 = dummy sinks for SRC_1/SRC_1_HI (consumed, never read)
_L_X, _L_W, _L_XH, _L_HL, _L_Z = 0, 1, 2, 3, 4
_L_D1, _L_D2 = 4, 5

_PD = AluInp.PREV_DELAY_0  # + lane id
_PA = AluInp.PREV_ALU_OUT


def _seed() -> UopConfig:
    """Zero the a-flops at stages 1..5 (superset of both ops' state flops)."""
    u = UopConfig()
    u.enable_input(InpSel.ZERO, _L_Z + 1)
    u.require_inp0 = 0
    u.repeat_count = 2
    u.trigger = (Trigger.COUNT, Trigger.NONE, Trigger.NONE)
    u.next_uop = (1, 0, 0)
    dp = u.datapath_config
    for k in range(5):
        dp[k].pass_through_delay(_L_Z)
    for k in range(1, 6):
        b = dp[k]
        b.op = AluOp.BYPASS
        b.alu_src0 = _PD + _L_Z
        b.alu_src1 = b.alu_src0
        b.alu_out_enable = ENABLE
        b.alu_out_a_enable = ENABLE
    return u


def _steady_base(next_idx: int, two_x: bool) -> UopConfig:
    u = UopConfig()
    u.enable_input(InpSel.SRC_0, _L_X + 1)
    u.enable_input(InpSel.CONST_0, _L_W + 1)
    u.enable_input(InpSel.SRC_1, _L_D1 + 1)
    if two_x:
        u.enable_input(InpSel.SRC_0_HI, _L_XH + 1)
        u.enable_input(InpSel.SRC_1_HI, _L_D2 + 1)
    u.require_inp0 = 1
    u.require_inp1 = 1
    u.repeat_count = 1
    u.trigger = (Trigger.SRC_TENSOR_DONE, Trigger.COUNT, Trigger.NONE)
    u.next_uop = (0, next_idx, 0)
    return u


def _chain_alu(u: UopConfig, from_blk: int):
    for k in range(from_blk, 8):
        u.datapath_config[k].pass_through_alu()
    u.enable_output(OutSel.ALU_OUT, OutPath.WR0_LO)


def _lscan_1x(mul: int, state: int, next_idx: int) -> UopConfig:
    u = _steady_base(next_idx, two_x=False)
    dp = u.datapath_config
    for k in range(mul):
        dp[k].pass_through_delay(_L_X, _L_W)
    dp[mul].enable_alu(AluOp.MULTIPLY, _PD + _L_W, AluInp.NEXT_ALU_OUT_A)
    dp[mul].pass_through_delay(_L_X)
    add = dp[state].enable_alu(AluOp.ADD, _PA, _PD + _L_X)
    add.alu_out_a_enable = ENABLE
    _chain_alu(u, state + 1)
    return u


def _lscan_2x(lo_mul: int, next_idx: int) -> UopConfig:
    u = _steady_base(next_idx, two_x=True)
    dp = u.datapath_config
    m0, a0 = lo_mul, lo_mul + 1
    m1, a1 = lo_mul + 2, lo_mul + 3
    for k in range(m0):
        dp[k].pass_through_delay(_L_X, _L_W, _L_XH)
    dp[m0].enable_alu(AluOp.MULTIPLY, _PD + _L_W, AluInp.NEXT_ALU_OUT_A)
    dp[m0].pass_through_delay(_L_X, _L_W, _L_XH)
    addlo = dp[a0].enable_alu(AluOp.ADD, _PA, _PD + _L_X)
    addlo.alu_out_a_enable = ENABLE
    addlo.pass_through_delay(_L_W, _L_XH)
    dp[m1].enable_alu(AluOp.MULTIPLY, _PD + _L_W, AluInp.NEXT_ALU_OUT_A)
    dp[m1].enable_delay_from_src(DelayInp.PREV_ALU_OUT, _L_HL)
    dp[m1].pass_through_delay(_L_XH)
    addhi = dp[a1].enable_alu(AluOp.ADD, _PA, _PD + _L_XH)
    addhi.alu_out_a_enable = ENABLE
    addhi.pass_through_delay(_L_HL)
    for k in range(a1 + 1, 8):
        dp[k].pass_through_alu()
        dp[k].pass_through_delay(_L_HL)
    u.enable_output(OutSel(int(OutSel.DELAY_0) + _L_HL), OutPath.WR0_LO)
    u.enable_output(OutSel.ALU_OUT, OutPath.WR0_HI)
    return u


def _qrec_1x(mul: int, next_idx: int) -> UopConfig:
    u = _steady_base(next_idx, two_x=False)
    dp = u.datapath_config
    for k in range(mul):
        dp[k].pass_through_delay(_L_X, _L_W)
    dp[mul].enable_alu(AluOp.MULTIPLY, _PD + _L_W, AluInp.NEXT_ALU_OUT_A)
    dp[mul].pass_through_delay(_L_X)
    pmax = dp[mul + 1].enable_alu(AluOp.MAX, _PA, _PD + _L_X)
    pmax.alu_out_a_enable = ENABLE
    pmax.pass_through_delay(_L_X)
    dp[mul + 2].enable_alu(AluOp.SUBTRACT, _PA, _PD + _L_X)
    _chain_alu(u, mul + 3)
    if mul + 1 == 6:
        u.accum_enabled = 1
    return u


def _qrec_2x(lo_mul: int, next_idx: int) -> UopConfig:
    u = _steady_base(next_idx, two_x=True)
    dp = u.datapath_config
    m = lo_mul
    for k in range(m):
        dp[k].pass_through_delay(_L_X, _L_W, _L_XH)
    dp[m].enable_alu(AluOp.MULTIPLY, _PD + _L_W, AluInp.NEXT_ALU_OUT_A)
    dp[m].pass_through_delay(_L_X, _L_W, _L_XH)
    pmax = dp[m + 1].enable_alu(AluOp.MAX, _PA, _PD + _L_X)
    pmax.alu_out_a_enable = ENABLE
    pmax.pass_through_delay(_L_X, _L_W, _L_XH)
    dp[m + 2].enable_alu(AluOp.SUBTRACT, _PA, _PD + _L_X)
    dp[m + 2].pass_through_delay(_L_W, _L_XH)
    dp[m + 3].enable_alu(AluOp.MULTIPLY, _PD + _L_W, AluInp.NEXT_ALU_OUT_A)
    dp[m + 3].enable_delay_from_src(DelayInp.PREV_ALU_OUT, _L_HL)
    dp[m + 3].pass_through_delay(_L_XH)
    pmaxh = dp[m + 4].enable_alu(AluOp.MAX, _PA, _PD + _L_XH)
    pmaxh.alu_out_a_enable = ENABLE
    pmaxh.pass_through_delay(_L_XH, _L_HL)
    dp[m + 5].enable_alu(AluOp.SUBTRACT, _PA, _PD + _L_XH)
    dp[m + 5].pass_through_delay(_L_HL)
    for k in range(m + 6, 8):
        dp[k].pass_through_alu()
        dp[k].pass_through_delay(_L_HL)
    u.enable_output(OutSel(int(OutSel.DELAY_0) + _L_HL), OutPath.WR0_LO)
    u.enable_output(OutSel.ALU_OUT, OutPath.WR0_HI)
    return u


def _lscan4_uops():
    # state flops: A@1, B@3, C@2, D@4
    return [
        _seed(),
        _lscan_1x(0, 1, next_idx=2),
        _lscan_1x(2, 3, next_idx=3),
        _lscan_1x(1, 2, next_idx=4),
        _lscan_1x(3, 4, next_idx=1),
    ]


def _lscan4_uops_2x():
    return [_seed(), _lscan_2x(0, next_idx=2), _lscan_2x(1, next_idx=1),
            UopConfig(), UopConfig()]


def _qrec4_uops():
    # state flops: A@1, B@4, C@2, D@5
    return [
        _seed(),
        _qrec_1x(0, next_idx=2),
        _qrec_1x(3, next_idx=3),
        _qrec_1x(1, next_idx=4),
        _qrec_1x(4, next_idx=1),
    ]


def _qrec4_uops_2x():
    return [_seed(), _qrec_2x(0, next_idx=2), _qrec_2x(1, next_idx=1),
            UopConfig(), UopConfig()]


def _ref_lscan4(in0, in1, c0, c1, c2):
    x = np.asarray(in0, np.float32).reshape(in0.shape[0], -1, 4)
    w = np.asarray(c0, np.float32).reshape(-1)
    out = np.empty_like(x)
    for s in range(4):
        acc = np.zeros_like(w)
        for t in range(x.shape[1]):
            acc = w * acc + x[:, t, s]
            out[:, t, s] = acc
    return out.reshape(in0.shape)


def _ref_qrec4(in0, in1, c0, c1, c2):
    el = np.asarray(in0, np.float32).reshape(in0.shape[0], -1, 4)
    w = np.asarray(c0, np.float32).reshape(-1)
    out = np.empty_like(el)
    for s in range(4):
        P_ = np.zeros_like(w)
        for t in range(el.shape[1]):
            v = w * P_
            P_ = np.maximum(el[:, t, s], v)
            out[:, t, s] = P_ - el[:, t, s]
    return out.reshape(in0.shape)


class _HandOp:
    """Duck-typed DveOp with hand-built uop programs (bypasses lower())."""

    def __init__(self, name, spec, uops, uops_2x, perf_max):
        self.name = name
        self.spec = spec
        self.subdim = False
        self._uops = uops
        self._uops_2x = uops_2x
        self._perf_max = perf_max
        self._cache = {}

    def compile(self, ver):
        assert ver == "v3", f"hand-built uops are v3-only, got {ver}"
        if ver not in self._cache:
            s = DveOpSpec(
                name=self.name,
                opcode=dve_ops.get_dve_sub_opcode(self.name),
                uops=self._uops,
                uops_2x=self._uops_2x,
                perf_max=self._perf_max,
                rd1_en=True,
            )
            s.validate(ver)
            self._cache[ver] = s
        return self._cache[ver]


def _register():
    import sys, types

    modname = f"ant_irnn4_ops_{_REV}"
    mod = sys.modules.get(modname)
    if mod is not None:
        return mod.LSCAN4, mod.QREC4
    spec1 = Spec(body=sp_relu(Src0 * C0), reference=_ref_lscan4)
    spec2 = Spec(body=sp_relu(Src0 * C0), reference=_ref_qrec4)
    ls = _HandOp(f"ANT_LSCAN4_{_REV}", spec1, _lscan4_uops(),
                 _lscan4_uops_2x(), perf_max=1)
    qr = _HandOp(f"ANT_QREC4_{_REV}", spec2, _qrec4_uops(),
                 _qrec4_uops_2x(), perf_max=1)
    base = max(dve_ops._SUB_OPCODE_FOR_NAME.values())
    dve_ops._SUB_OPCODE_FOR_NAME[ls.name] = base + 1
    dve_ops._SUB_OPCODE_FOR_NAME[qr.name] = base + 2
    assert max(dve_ops._SUB_OPCODE_FOR_NAME.values()) < 0x20
    dve_ops.OPS.append(ls)
    dve_ops.OPS.append(qr)
    dve_ops.CUSTOM_DVE_SPECS[ls.name] = ls.spec
    dve_ops.CUSTOM_DVE_SPECS[qr.name] = qr.spec
    mod = types.ModuleType(modname)
    mod.LSCAN4, mod.QREC4 = ls, qr
    sys.modules[modname] = mod
    return ls, qr


LSCAN4, QREC4 = _register()

# --- kernel ---

from contextlib import ExitStack

import concourse.bass as bass
import concourse.tile as tile
from concourse import mybir
from concourse.bass_utils import run_bass_kernel_spmd


dt = mybir.dt
Act = mybir.ActivationFunctionType

B, T, I, H, L = 32, 2048, 256, 512, 4
NCORES = 8
BLOC = B // NCORES          # 4 batches per core = one interleave quad
P = 128
TCH = 512
M4 = H // P
KI = I // P
QT = 4 * T                  # quad-interleaved stream length


def build(include_bias=False):
    nc = bass.Bass("TRN2", target_bir_lowering=False, debug=False,
                   num_devices=NCORES)
    # pair-interleaved input: xT[p, i, 2t+j] = x[2p+j, t, i]
    xT_d = nc.dram_tensor("xT", [2, I, 2 * T], dt.float16, kind="ExternalInput").ap()
    w0_d = nc.dram_tensor("w0tn", [I, H], dt.float16, kind="ExternalInput").ap()
    ws_d = nc.dram_tensor("wstn", [L - 1, H, H], dt.float16, kind="ExternalInput").ap()
    wq_d = nc.dram_tensor("wq", [P, L * M4], dt.float32, kind="ExternalInput").ap()
    bias_d = nc.dram_tensor("biasn", [L, 1, H], dt.float16, kind="ExternalInput").ap()
    # quad-interleaved output: [H, (t b)] -- host de-interleaves
    out_d = nc.dram_tensor("out", [H, QT], dt.float16,
                           kind="ExternalOutput").ap()

    with tile.TileContext(nc) as tc, ExitStack() as ctx:
        wpool = ctx.enter_context(tc.tile_pool(name="weights", bufs=1))
        xpool = ctx.enter_context(tc.tile_pool(name="xin", bufs=1))
        spool = ctx.enter_context(tc.tile_pool(name="stage", bufs=1))
        psum = ctx.enter_context(tc.tile_pool(name="psum", bufs=2, space="PSUM"))

        # ---- persistent weights ----
        wq_dmas, crit_dmas, late_dmas = [], [], []
        wqall = wpool.tile([P, L * M4], dt.float32, tag="wqall")
        wq_dmas.append(nc.gpsimd.dma_start(out=wqall[:], in_=wq_d))
        wq = [[wqall[:, (l * M4 + m):(l * M4 + m) + 1] for m in range(M4)]
              for l in range(L)]
        wt = [[] for _ in range(L)]
        for k in range(KI):
            w = wpool.tile([P, H], dt.float16, tag=f"w0{k}")
            crit_dmas.append(nc.gpsimd.dma_start(
                out=w[:], in_=w0_d[k * P:(k + 1) * P, :]))
            wt[0].append(w)
        # input tiles (pair-interleaved): pair 0 split into 4 chunks so the
        # pipeline can start on chunk 0, pair 1 full-tile afterwards
        xin = [[] for _ in range(2)]
        xin_dmas = [[] for _ in range(2)]
        for p in range(2):
            for k in range(KI):
                xt = xpool.tile([P, 2 * T], dt.float16, tag=f"x{p}{k}")
                xin[p].append(xt)
        for c in range(8):
            cs = slice(c * 512, (c + 1) * 512)
            for k in range(KI):
                d = nc.gpsimd.dma_start(out=xin[0][k][:, cs],
                                        in_=xT_d[0, k * P:(k + 1) * P, cs])
                crit_dmas.append(d)
                xin_dmas[0].append((c, d))
        for c in range(4):
            cs = slice(c * 1024, (c + 1) * 1024)
            for k in range(KI):
                d = nc.gpsimd.dma_start(out=xin[1][k][:, cs],
                                        in_=xT_d[1, k * P:(k + 1) * P, cs])
                late_dmas.append(d)
                xin_dmas[1].append((c, d))
        ws_dmas = []
        for l in range(1, L):
            for k in range(M4):
                w = wpool.tile([P, H], dt.float16, tag=f"w{l}{k}")
                d = nc.gpsimd.dma_start(
                    out=w[:], in_=ws_d[l - 1, k * P:(k + 1) * P, :])
                ws_dmas.append(d)
                wt[l].append(w)
        bias = None
        if include_bias:
            bias = []
            for l in range(L):
                bt = wpool.tile([1, H], dt.float16, tag=f"b{l}")
                ws_dmas.append(nc.gpsimd.dma_start(out=bt[:],
                                                   in_=bias_d[l, :, :]))
                bias.append(bt)
            ones = wpool.tile([1, TCH], dt.float16, tag="ones")
            nc.gpsimd.memset(ones[:], 1.0)

        # ---- absorber machinery (per-engine pinned chains) ----
        scr_v = wpool.tile([P, 160], dt.float32, tag="scr_v")
        scr_a = wpool.tile([P, 160], dt.float32, tag="scr_a")
        state = {"V": [None, 0], "A": [None, 0], "PE": [None]}

        def absorb(eng, dep=None):
            if eng == "V":
                prev, k = state[eng]
                i = nc.vector.tensor_copy(scr_v[:, k:k + 1], wq[0][0][:])
            elif eng == "A":
                prev, k = state[eng]
                i = nc.scalar.activation(scr_a[:, k:k + 1], wq[0][0][:],
                                         Act.Copy)
            else:
                prev = state[eng][0]
                # 1-column load: cheapest PE instruction that can carry a wait
                i = nc.tensor.ldweights(weights=wt[0][0][:, 0:1])
            if prev is not None:
                bass._add_dep_helper(i.ins, prev.ins, sync=False, reason="chain")
            if dep is not None:
                bass._add_dep_helper(i.ins, dep.ins, sync=True, reason="absorb")
            if eng == "PE":
                state[eng] = [i]
            else:
                state[eng] = [i, (state[eng][1] + 1) % 160]
            return i

        def pin(real, eng):
            prev = state[eng][0]
            if prev is not None:
                bass._add_dep_helper(real.ins, prev.ins, sync=False, reason="pin")
            state[eng][0] = real
            return real

        # warm-up: junk compute while input DMAs land (p-state ramp)
        jw = wpool.tile([P, P], dt.float16, tag="jw")
        jx = wpool.tile([P, 2048], dt.float16, tag="jx")
        jo = wpool.tile([P, 2048], dt.float16, tag="jo")
        jq = wpool.tile([P, 1], dt.float32, tag="jq")
        nc.vector.memset(jw[:, 0:1], 0.125)
        nc.vector.memset(jx[:, 0:1], 0.125)
        nc.vector.memset(jq[:], 0.125)
        for s in range(2):
            jp = psum.tile([P, T], dt.float32, tag="xp")
            for r in range(12):
                pin(nc.tensor.matmul(jp[:, 0:P], lhsT=jw[:],
                                     rhs=jx[:, 0:P],
                                     start=True, stop=True), "PE")
        for r in range(3):
            ji = nc.vector._custom_dve(LSCAN4, out=jo[:], in0=jx[:],
                                       in1=jx[:], s0=jq[:])
            ji.ins.perf_max = 1
            pin(ji, "V")

        # engine init
        for eng in ("V", "A"):
            absorb(eng)
            for d in wq_dmas:
                absorb(eng, d)
        for k in range(KI):
            pin(nc.tensor.ldweights(weights=wt[0][k][:, 0:P]), "PE")
        if include_bias:
            for l in range(L):
                pin(nc.tensor.ldweights(weights=bias[l][:, 0:P]), "PE")
            pin(nc.tensor.ldweights(weights=ones[:, 0:P]), "PE")

        # ---- main loop ----
        NXSB, NHP = 2, 7
        hp_ring = []        # (l, m) keys
        hp_by_key = {}
        hp_readers = {}     # key -> last matmul reading it
        xsb_readers = [None] * NXSB   # ring idx -> last QREC4 reader
        psum_readers = [None, None]   # psum slot -> ACT copy that read it
        xsb_i = 0
        psum_i = 0
        hp_i = 0
        last_qrec = None
        qrec_by = {}
        out_tiles = []  # layer-3 output tiles; fake end-of-trace readers keep
                        # them live so the allocator never hands a DMA-read
                        # buffer to a later tile (a DMA-completion release dep
                        # would exceed walrus's 1-wait budget)

        def mm_rhs(l, k, p, h, n):
            if l == 0:
                return xin[p][k][:, h * 2048 + n * TCH:
                                 h * 2048 + (n + 1) * TCH]
            t0 = h * 1024 + n * 256
            return hp_by_key[(l - 1, k)][:].rearrange(
                "p (t s) -> p t s", s=4)[:, t0:t0 + 256, 2 * p:2 * p + 2]

        def emit_mms(l, m, p, h, xp, ks, stop_k):
            """Emit the k in `ks` matmuls for all 4 chunks of group (p, h).
            stop_k: the k that carries stop (None while pre-issuing)."""
            kprev = KI if l == 0 else M4
            last_mm = None
            for n in range(4):  # 256-timestep chunks
                ns = slice(n * TCH, (n + 1) * TCH)
                if l == 0 and m == 0:
                    c_need = h * 4 + n if p == 0 else h * 2 + n // 2
                    for c, d in xin_dmas[p]:
                        if c == c_need:
                            absorb("PE", d)
                for k in ks:
                    last_mm = nc.tensor.matmul(
                        xp[:, ns], lhsT=wt[l][k][:, m * P:(m + 1) * P],
                        rhs=mm_rhs(l, k, p, h, n), start=(k == 0),
                        stop=(k == stop_k and not include_bias))
                    pin(last_mm, "PE")
                    if l > 0:
                        hp_readers[(l - 1, k)] = last_mm
                if include_bias and stop_k is not None and stop_k in ks:
                    last_mm = pin(nc.tensor.matmul(
                        xp[:, ns], lhsT=bias[l][:, m * P:(m + 1) * P],
                        rhs=ones[:, :], start=False, stop=True), "PE")
            return last_mm

        for l in range(L):
            kprev = KI if l == 0 else M4
            if l > 0:
                for k in range(M4):
                    pin(nc.tensor.ldweights(weights=wt[l][k][:, 0:P]), "PE")
            for m in range(M4):
                xsb = spool.tile([P, QT], dt.float16, tag="xsb", bufs=NXSB)
                xsb_slot = xsb_i % NXSB
                xsb_i += 1
                last_cp = None
                # one matmul computes a batch PAIR's chunk: moving columns
                # are pair-interleaved, so PSUM holds interleaved xp pairs
                # and ACT writes 2-contiguous/skip-2 into the quad tile.
                # At a layer boundary, pre-issue k<last for the first two
                # groups before waiting on the previous layer's final QREC4
                # so PE stays busy (and HAM stays warm) through the handoff.
                pre = {}
                if l > 0 and m == 0:
                    # hp(l-1, 0..2) finished long ago; absorbing their QREC4s
                    # lets the k<3 pre-issue run without foreign waits
                    absorb("PE", qrec_by[(l - 1, 2)])
                    for p, h in ((0, 0), (0, 1)):
                        xp = psum.tile([P, T], dt.float32, tag="xp")
                        slot = psum_i % 2
                        psum_i += 1
                        old_rd = psum_readers[slot]
                        if old_rd is not None:
                            absorb("PE", old_rd)
                        emit_mms(l, m, p, h, xp, range(kprev - 1), None)
                        pre[(p, h)] = (xp, slot)
                    absorb("PE", last_qrec)
                for p in range(2):
                    for h in range(2):
                        if (p, h) in pre:
                            xp, slot = pre[(p, h)]
                            last_mm = emit_mms(l, m, p, h, xp,
                                               [kprev - 1], kprev - 1)
                        else:
                            xp = psum.tile([P, T], dt.float32, tag="xp")
                            slot = psum_i % 2
                            psum_i += 1
                            old_rd = psum_readers[slot]
                            if old_rd is not None:
                                absorb("PE", old_rd)
                            last_mm = emit_mms(l, m, p, h, xp,
                                               range(kprev), kprev - 1)
                        # ACT: PSUM pair-interleaved fp32 -> xsb quad (fp16)
                        old_x = xsb_readers[xsb_slot]
                        if p == 0 and h == 0 and old_x is not None:
                            absorb("A", old_x)
                        absorb("A", last_mm)
                        cp = pin(nc.scalar.activation(
                            xsb[:].rearrange("p (t s) -> p t s", s=4)[
                                :, h * 1024:(h + 1) * 1024, 2 * p:2 * p + 2],
                            xp[:].rearrange("p (t s) -> p t s", s=2),
                            Act.Copy), "A")
                        psum_readers[slot] = cp
                        last_cp = cp
                # DVE: LSCAN4 in-place on xsb, then QREC4 -> hp tile
                absorb("V", last_cp)
                ls = nc.vector._custom_dve(LSCAN4, out=xsb[:], in0=xsb[:],
                                           in1=xsb[:], s0=wq[l][m][:])
                ls.ins.perf_max = 1
                pin(ls, "V")
                hpt = spool.tile([P, QT], dt.float16, tag="hp", bufs=NHP)
                if len(hp_ring) >= NHP:
                    old_key = hp_ring[hp_i % NHP]
                    rd = hp_readers.pop(old_key, None)
                    if rd is not None:
                        absorb("V", rd)
                if len(hp_ring) < NHP:
                    hp_ring.append((l, m))
                else:
                    hp_ring[hp_i % NHP] = (l, m)
                hp_i += 1
                hp_by_key[(l, m)] = hpt
                qr = nc.vector._custom_dve(QREC4, out=hpt[:], in0=xsb[:],
                                           in1=xsb[:], s0=wq[l][m][:])
                qr.ins.perf_max = 1
                pin(qr, "V")
                xsb_readers[xsb_slot] = qr
                last_qrec = qr
                qrec_by[(l, m)] = qr
                if l == L - 1:
                    # 2 chunks per tile: 8 sync-queue DMAs total (the 9th
                    # would pick up a flow-control wait and trip walrus's
                    # 1-wait budget)
                    for c in range(2):
                        cs = slice(c * (QT // 2), (c + 1) * (QT // 2))
                        nc.sync.dma_start(
                            out=out_d[m * P:(m + 1) * P, cs],
                            in_=hpt[:, cs])
                    out_tiles.append(hpt)

        # fake readers: pin the DMA'd tiles live until end-of-trace
        for ht in out_tiles:
            pin(nc.vector.tensor_copy(scr_v[:, state["V"][1]:state["V"][1] + 1],
                                      ht[:, 0:1]), "V")
            state["V"][1] = (state["V"][1] + 1) % 160

        # ---- tail pre-drains ----
        tail_deps = [i for i in nc.inst_map.values()
                     if type(i).__name__ == "InstDMACopy"]
        snap = list(nc.inst_map.values())
        for eng in ("DVE", "Activation", "PE"):
            last_e = [i for i in snap
                      if str(getattr(i, "engine", "")).endswith(eng)]
            if last_e:
                tail_deps.append(last_e[-1])
        for depi in tail_deps:
            dr = nc.sync.drain(fusable=False)
            bass._add_dep_helper(dr.ins, depi, sync=True,
                                 reason="tail pre-drain absorber")
    assert mybir.codegen_inst_isa_subclasses(nc)
    _assert_wait_budget(nc)
    return nc


_MULTI_WAIT_OK = {"InstDrain",
                  "InstEventSemaphore", "InstUnconditionalBranch",
                  "InstRegisterMove", "InstISA", "InstTensorLoad",
                  "InstTensorSave"}


def _assert_wait_budget(nc):
    bad = []
    for name, inst in nc.inst_map.items():
        ty = type(inst).__name__
        w = inst.sync_info.on_wait if inst.sync_info else []
        if ty == "InstCustomDveAnt":
            fw = [x for x in w if not x.ant_name.startswith("DVE")]
            if fw:
                bad.append((name, ty, [f"{x.ant_name}>={x.wait_value}"
                                       for x in fw]))
            continue
        if ty in _MULTI_WAIT_OK:
            continue
        if len(w) > 1:
            bad.append((name, ty,
                        [f"{x.ant_name}>={x.wait_value}" for x in w]))
    if bad:
        raise RuntimeError(
            f"{len(bad)} instructions exceed the sync-wait budget, "
            f"first few: {bad[:6]}")


def _prep_core_inputs(Input, W0, Ws, bs, whs, core):
    bsl = slice(core * BLOC, (core + 1) * BLOC)
    xb = Input[bsl]                      # [4, T, I]
    # pair-interleave: xT[p, i, 2t+j] = xb[2p+j, t, i]
    xT = np.ascontiguousarray(
        xb.reshape(2, 2, T, I).transpose(0, 3, 2, 1).reshape(2, I, 2 * T))
    return {
        "xT": xT.astype(np.float16),
        "w0tn": np.ascontiguousarray(-W0.T).astype(np.float16),
        "wstn": np.ascontiguousarray(-Ws.transpose(0, 2, 1)).astype(np.float16),
        "wq": np.ascontiguousarray(
            whs.astype(np.float32).reshape(L, M4, P).transpose(2, 0, 1)
            .reshape(P, L * M4)),
        "biasn": np.ascontiguousarray(-bs[:, None, :]).astype(np.float16),
    }


def kernel(Input, W0, Ws, bs, whs):
    include_bias = bool(np.any(bs != 0))
    nc = build(include_bias=include_bias)
    in_maps = [_prep_core_inputs(Input, W0, Ws, bs, whs, r)
               for r in range(NCORES)]
    res = run_bass_kernel_spmd(nc, in_maps, core_ids=list(range(NCORES)))
    parts = []
    for r in range(NCORES):
        o = res.results[r]["out"]  # [H, 4T] quad-interleaved
        o = o.reshape(H, T, BLOC).transpose(2, 0, 1)  # [BLOC, H, T]
        parts.append(o)
    full = np.concatenate(parts, axis=0)  # [B, H, T]
    return np.ascontiguousarray(full.transpose(0, 2, 1)).astype(np.float32)


# revision 31
# speedup vs baseline: 1.9839x; 1.0653x over previous
"""Trainium2 Bass kernel for a 4-layer IndRNN (B=32, T=2048, I=256, H=512).

v3: 4-stream interleaved custom DVE ops (ANT_LSCAN4 / ANT_QREC4) with
2X_1PORT uop programs run the whole recurrence at 2 fp16 elem/cycle.
All 4 batches of a core are element-interleaved (a_t,b_t,c_t,d_t,...) in
one [128, 4T] fp16 stream per (layer, m-tile); one LSCAN4 + one QREC4
per group replaces the baseline's four pair ops at half the DVE time.

Math: per layer, with PSUM holding -xp (weights negated on host):
    l_t = w*l_{t-1} + (-xp_t)                     (LSCAN4; in-place)
    v_t = w*P_{t-1}; P_t = max(l_t, v_t); h_t = P_t - l_t   (QREC4)
which equals h_t = relu(xp_t + w*h_{t-1}), the IndRNN layer
(P - l == max(v - l, 0) saves an ALU stage -> 3 ops/elem, so two
elements fit the 8-stage datapath in 2x mode).

2x mode notes: rd1_en=1 with in1 := in0 forces the handler's TwoSrc perf
enable so only 2X_1PORT is reachable (2X_2PORT/4X would feed the uops a
port layout they can't drain -> engine hang); the uops consume SRC_1 into
dummy lanes. perf_max=1 is set on each instruction (byte-36[7:6]).

Sharding: data-parallel over batch, 4 batches (= 1 quad) per core.
"""

import numpy as np

from concourse import dve_ops
from concourse.dve_spec import Spec, Src0, C0, relu as sp_relu
from concourse.dve_uop import (
    AluInp,
    AluOp,
    DelayInp,
    DveOpSpec,
    ENABLE,
    InpSel,
    OutPath,
    OutSel,
    Trigger,
    UopConfig,
)

_REV = "r4"

# lanes: X=SRC_0(LO), W=CONST_0, XH=SRC_0_HI, HL=LO-result carry (2x),
# D1/D2 = dummy sinks for SRC_1/SRC_1_HI (consumed, never read)
_L_X, _L_W, _L_XH, _L_HL, _L_Z = 0, 1, 2, 3, 4
_L_D1, _L_D2 = 4, 5

_PD = AluInp.PREV_DELAY_0  # + lane id
_PA = AluInp.PREV_ALU_OUT


def _seed() -> UopConfig:
    """Zero the a-flops at stages 1..5 (superset of both ops' state flops)."""
    u = UopConfig()
    u.enable_input(InpSel.ZERO, _L_Z + 1)
    u.require_inp0 = 0
    u.repeat_count = 2
    u.trigger = (Trigger.COUNT, Trigger.NONE, Trigger.NONE)
    u.next_uop = (1, 0, 0)
    dp = u.datapath_config
    for k in range(5):
        dp[k].pass_through_delay(_L_Z)
    for k in range(1, 6):
        b = dp[k]
        b.op = AluOp.BYPASS
        b.alu_src0 = _PD + _L_Z
        b.alu_src1 = b.alu_src0
        b.alu_out_enable = ENABLE
        b.alu_out_a_enable = ENABLE
    return u


def _steady_base(next_idx: int, two_x: bool) -> UopConfig:
    u = UopConfig()
    u.enable_input(InpSel.SRC_0, _L_X + 1)
    u.enable_input(InpSel.CONST_0, _L_W + 1)
    u.enable_input(InpSel.SRC_1, _L_D1 + 1)
    if two_x:
        u.enable_input(InpSel.SRC_0_HI, _L_XH + 1)
        u.enable_input(InpSel.SRC_1_HI, _L_D2 + 1)
    u.require_inp0 = 1
    u.require_inp1 = 1
    u.repeat_count = 1
    u.trigger = (Trigger.SRC_TENSOR_DONE, Trigger.COUNT, Trigger.NONE)
    u.next_uop = (0, next_idx, 0)
    return u


def _chain_alu(u: UopConfig, from_blk: int):
    for k in range(from_blk, 8):
        u.datapath_config[k].pass_through_alu()
    u.enable_output(OutSel.ALU_OUT, OutPath.WR0_LO)


def _lscan_1x(mul: int, state: int, next_idx: int) -> UopConfig:
    u = _steady_base(next_idx, two_x=False)
    dp = u.datapath_config
    for k in range(mul):
        dp[k].pass_through_delay(_L_X, _L_W)
    dp[mul].enable_alu(AluOp.MULTIPLY, _PD + _L_W, AluInp.NEXT_ALU_OUT_A)
    dp[mul].pass_through_delay(_L_X)
    add = dp[state].enable_alu(AluOp.ADD, _PA, _PD + _L_X)
    add.alu_out_a_enable = ENABLE
    _chain_alu(u, state + 1)
    return u


def _lscan_2x(lo_mul: int, next_idx: int) -> UopConfig:
    u = _steady_base(next_idx, two_x=True)
    dp = u.datapath_config
    m0, a0 = lo_mul, lo_mul + 1
    m1, a1 = lo_mul + 2, lo_mul + 3
    for k in range(m0):
        dp[k].pass_through_delay(_L_X, _L_W, _L_XH)
    dp[m0].enable_alu(AluOp.MULTIPLY, _PD + _L_W, AluInp.NEXT_ALU_OUT_A)
    dp[m0].pass_through_delay(_L_X, _L_W, _L_XH)
    addlo = dp[a0].enable_alu(AluOp.ADD, _PA, _PD + _L_X)
    addlo.alu_out_a_enable = ENABLE
    addlo.pass_through_delay(_L_W, _L_XH)
    dp[m1].enable_alu(AluOp.MULTIPLY, _PD + _L_W, AluInp.NEXT_ALU_OUT_A)
    dp[m1].enable_delay_from_src(DelayInp.PREV_ALU_OUT, _L_HL)
    dp[m1].pass_through_delay(_L_XH)
    addhi = dp[a1].enable_alu(AluOp.ADD, _PA, _PD + _L_XH)
    addhi.alu_out_a_enable = ENABLE
    addhi.pass_through_delay(_L_HL)
    for k in range(a1 + 1, 8):
        dp[k].pass_through_alu()
        dp[k].pass_through_delay(_L_HL)
    u.enable_output(OutSel(int(OutSel.DELAY_0) + _L_HL), OutPath.WR0_LO)
    u.enable_output(OutSel.ALU_OUT, OutPath.WR0_HI)
    return u


def _qrec_1x(mul: int, next_idx: int) -> UopConfig:
    u = _steady_base(next_idx, two_x=False)
    dp = u.datapath_config
    for k in range(mul):
        dp[k].pass_through_delay(_L_X, _L_W)
    dp[mul].enable_alu(AluOp.MULTIPLY, _PD + _L_W, AluInp.NEXT_ALU_OUT_A)
    dp[mul].pass_through_delay(_L_X)
    pmax = dp[mul + 1].enable_alu(AluOp.MAX, _PA, _PD + _L_X)
    pmax.alu_out_a_enable = ENABLE
    pmax.pass_through_delay(_L_X)
    dp[mul + 2].enable_alu(AluOp.SUBTRACT, _PA, _PD + _L_X)
    _chain_alu(u, mul + 3)
    if mul + 1 == 6:
        u.accum_enabled = 1
    return u


def _qrec_2x(lo_mul: int, next_idx: int) -> UopConfig:
    u = _steady_base(next_idx, two_x=True)
    dp = u.datapath_config
    m = lo_mul
    for k in range(m):
        dp[k].pass_through_delay(_L_X, _L_W, _L_XH)
    dp[m].enable_alu(AluOp.MULTIPLY, _PD + _L_W, AluInp.NEXT_ALU_OUT_A)
    dp[m].pass_through_delay(_L_X, _L_W, _L_XH)
    pmax = dp[m + 1].enable_alu(AluOp.MAX, _PA, _PD + _L_X)
    pmax.alu_out_a_enable = ENABLE
    pmax.pass_through_delay(_L_X, _L_W, _L_XH)
    dp[m + 2].enable_alu(AluOp.SUBTRACT, _PA, _PD + _L_X)
    dp[m + 2].pass_through_delay(_L_W, _L_XH)
    dp[m + 3].enable_alu(AluOp.MULTIPLY, _PD + _L_W, AluInp.NEXT_ALU_OUT_A)
    dp[m + 3].enable_delay_from_src(DelayInp.PREV_ALU_OUT, _L_HL)
    dp[m + 3].pass_through_delay(_L_XH)
    pmaxh = dp[m + 4].enable_alu(AluOp.MAX, _PA, _PD + _L_XH)
    pmaxh.alu_out_a_enable = ENABLE
    pmaxh.pass_through_delay(_L_XH, _L_HL)
    dp[m + 5].enable_alu(AluOp.SUBTRACT, _PA, _PD + _L_XH)
    dp[m + 5].pass_through_delay(_L_HL)
    for k in range(m + 6, 8):
        dp[k].pass_through_alu()
        dp[k].pass_through_delay(_L_HL)
    u.enable_output(OutSel(int(OutSel.DELAY_0) + _L_HL), OutPath.WR0_LO)
    u.enable_output(OutSel.ALU_OUT, OutPath.WR0_HI)
    return u


def _lscan4_uops():
    # state flops: A@1, B@3, C@2, D@4
    return [
        _seed(),
        _lscan_1x(0, 1, next_idx=2),
        _lscan_1x(2, 3, next_idx=3),
        _lscan_1x(1, 2, next_idx=4),
        _lscan_1x(3, 4, next_idx=1),
    ]


def _lscan4_uops_2x():
    return [_seed(), _lscan_2x(0, next_idx=2), _lscan_2x(1, next_idx=1),
            UopConfig(), UopConfig()]


def _qrec4_uops():
    # state flops: A@1, B@4, C@2, D@5
    return [
        _seed(),
        _qrec_1x(0, next_idx=2),
        _qrec_1x(3, next_idx=3),
        _qrec_1x(1, next_idx=4),
        _qrec_1x(4, next_idx=1),
    ]


def _qrec4_uops_2x():
    return [_seed(), _qrec_2x(0, next_idx=2), _qrec_2x(1, next_idx=1),
            UopConfig(), UopConfig()]


# Continuation variants: no seed — a-flop states persist across the
# instruction boundary, resuming the recurrence from the previous
# instruction. uop[0] cannot be a loop target, so the entry steady uop is
# duplicated at the tail.


def _lscan4c_uops():
    return [
        _lscan_1x(0, 1, next_idx=1),
        _lscan_1x(2, 3, next_idx=2),
        _lscan_1x(1, 2, next_idx=3),
        _lscan_1x(3, 4, next_idx=4),
        _lscan_1x(0, 1, next_idx=1),
    ]


def _lscan4c_uops_2x():
    return [_lscan_2x(0, next_idx=1), _lscan_2x(1, next_idx=2),
            _lscan_2x(0, next_idx=1), UopConfig(), UopConfig()]


def _qrec4c_uops():
    return [
        _qrec_1x(0, next_idx=1),
        _qrec_1x(3, next_idx=2),
        _qrec_1x(1, next_idx=3),
        _qrec_1x(4, next_idx=4),
        _qrec_1x(0, next_idx=1),
    ]


def _qrec4c_uops_2x():
    return [_qrec_2x(0, next_idx=1), _qrec_2x(1, next_idx=2),
            _qrec_2x(0, next_idx=1), UopConfig(), UopConfig()]


def _ref_lscan4(in0, in1, c0, c1, c2):
    x = np.asarray(in0, np.float32).reshape(in0.shape[0], -1, 4)
    w = np.asarray(c0, np.float32).reshape(-1)
    out = np.empty_like(x)
    for s in range(4):
        acc = np.zeros_like(w)
        for t in range(x.shape[1]):
            acc = w * acc + x[:, t, s]
            out[:, t, s] = acc
    return out.reshape(in0.shape)


def _ref_qrec4(in0, in1, c0, c1, c2):
    el = np.asarray(in0, np.float32).reshape(in0.shape[0], -1, 4)
    w = np.asarray(c0, np.float32).reshape(-1)
    out = np.empty_like(el)
    for s in range(4):
        P_ = np.zeros_like(w)
        for t in range(el.shape[1]):
            v = w * P_
            P_ = np.maximum(el[:, t, s], v)
            out[:, t, s] = P_ - el[:, t, s]
    return out.reshape(in0.shape)


class _HandOp:
    """Duck-typed DveOp with hand-built uop programs (bypasses lower())."""

    def __init__(self, name, spec, uops, uops_2x, perf_max):
        self.name = name
        self.spec = spec
        self.subdim = False
        self._uops = uops
        self._uops_2x = uops_2x
        self._perf_max = perf_max
        self._cache = {}

    def compile(self, ver):
        assert ver == "v3", f"hand-built uops are v3-only, got {ver}"
        if ver not in self._cache:
            s = DveOpSpec(
                name=self.name,
                opcode=dve_ops.get_dve_sub_opcode(self.name),
                uops=self._uops,
                uops_2x=self._uops_2x,
                perf_max=self._perf_max,
                rd1_en=True,
            )
            s.validate(ver)
            self._cache[ver] = s
        return self._cache[ver]


def _register():
    import sys, types

    modname = f"ant_irnn4_ops_{_REV}"
    mod = sys.modules.get(modname)
    if mod is not None:
        return mod.LSCAN4, mod.QREC4, mod.LSCAN4C, mod.QREC4C
    spec1 = Spec(body=sp_relu(Src0 * C0), reference=_ref_lscan4)
    spec2 = Spec(body=sp_relu(Src0 * C0), reference=_ref_qrec4)
    ops = [
        _HandOp(f"ANT_LSCAN4_{_REV}", spec1, _lscan4_uops(),
                _lscan4_uops_2x(), perf_max=1),
        _HandOp(f"ANT_QREC4_{_REV}", spec2, _qrec4_uops(),
                _qrec4_uops_2x(), perf_max=1),
        _HandOp(f"ANT_LSCAN4C_{_REV}", spec1, _lscan4c_uops(),
                _lscan4c_uops_2x(), perf_max=1),
        _HandOp(f"ANT_QREC4C_{_REV}", spec2, _qrec4c_uops(),
                _qrec4c_uops_2x(), perf_max=1),
    ]
    base = max(dve_ops._SUB_OPCODE_FOR_NAME.values())
    for j, op in enumerate(ops):
        dve_ops._SUB_OPCODE_FOR_NAME[op.name] = base + 1 + j
        dve_ops.OPS.append(op)
        dve_ops.CUSTOM_DVE_SPECS[op.name] = op.spec
    assert max(dve_ops._SUB_OPCODE_FOR_NAME.values()) < 0x20
    mod = types.ModuleType(modname)
    mod.LSCAN4, mod.QREC4, mod.LSCAN4C, mod.QREC4C = ops
    sys.modules[modname] = mod
    return tuple(ops)


LSCAN4, QREC4, LSCAN4C, QREC4C = _register()

# --- kernel ---

from contextlib import ExitStack

import concourse.bass as bass
import concourse.tile as tile
from concourse import mybir
from concourse.bass_utils import run_bass_kernel_spmd


dt = mybir.dt
Act = mybir.ActivationFunctionType

B, T, I, H, L = 32, 2048, 256, 512, 4
NCORES = 8
BLOC = B // NCORES          # 4 batches per core = one interleave quad
P = 128
TCH = 512
M4 = H // P
KI = I // P
QT = 4 * T                  # quad-interleaved stream length


def build(include_bias=False):
    nc = bass.Bass("TRN2", target_bir_lowering=False, debug=False,
                   num_devices=NCORES)
    # pair-interleaved input: xT[p, i, 2t+j] = x[2p+j, t, i]
    xT_d = nc.dram_tensor("xT", [2, I, 2 * T], dt.float16, kind="ExternalInput").ap()
    w0_d = nc.dram_tensor("w0tn", [I, H], dt.float16, kind="ExternalInput").ap()
    ws_d = nc.dram_tensor("wstn", [L - 1, H, H], dt.float16, kind="ExternalInput").ap()
    wq_d = nc.dram_tensor("wq", [P, L * M4], dt.float32, kind="ExternalInput").ap()
    bias_d = nc.dram_tensor("biasn", [L, 1, H], dt.float16, kind="ExternalInput").ap()
    # quad-interleaved output: [H, (t b)] -- host de-interleaves
    out_d = nc.dram_tensor("out", [H, QT], dt.float16,
                           kind="ExternalOutput").ap()

    with tile.TileContext(nc) as tc, ExitStack() as ctx:
        wpool = ctx.enter_context(tc.tile_pool(name="weights", bufs=1))
        xpool = ctx.enter_context(tc.tile_pool(name="xin", bufs=1))
        spool = ctx.enter_context(tc.tile_pool(name="stage", bufs=1))
        psum = ctx.enter_context(tc.tile_pool(name="psum", bufs=2, space="PSUM"))

        # ---- persistent weights ----
        wq_dmas, crit_dmas, late_dmas = [], [], []
        wqall = wpool.tile([P, L * M4], dt.float32, tag="wqall")
        wq_dmas.append(nc.gpsimd.dma_start(out=wqall[:], in_=wq_d))
        wq = [[wqall[:, (l * M4 + m):(l * M4 + m) + 1] for m in range(M4)]
              for l in range(L)]
        wt = [[] for _ in range(L)]
        for k in range(KI):
            w = wpool.tile([P, H], dt.float16, tag=f"w0{k}")
            crit_dmas.append(nc.gpsimd.dma_start(
                out=w[:], in_=w0_d[k * P:(k + 1) * P, :]))
            wt[0].append(w)
        # input tiles (pair-interleaved): pair 0 split into 4 chunks so the
        # pipeline can start on chunk 0, pair 1 full-tile afterwards
        xin = [[] for _ in range(2)]
        xin_dmas = [[] for _ in range(2)]
        for p in range(2):
            for k in range(KI):
                xt = xpool.tile([P, 2 * T], dt.float16, tag=f"x{p}{k}")
                xin[p].append(xt)
        for c in range(8):
            cs = slice(c * 512, (c + 1) * 512)
            for k in range(KI):
                d = nc.gpsimd.dma_start(out=xin[0][k][:, cs],
                                        in_=xT_d[0, k * P:(k + 1) * P, cs])
                crit_dmas.append(d)
                xin_dmas[0].append((c, d))
        for c in range(4):
            cs = slice(c * 1024, (c + 1) * 1024)
            for k in range(KI):
                d = nc.gpsimd.dma_start(out=xin[1][k][:, cs],
                                        in_=xT_d[1, k * P:(k + 1) * P, cs])
                late_dmas.append(d)
                xin_dmas[1].append((c, d))
        ws_dmas = []
        for l in range(1, L):
            for k in range(M4):
                w = wpool.tile([P, H], dt.float16, tag=f"w{l}{k}")
                d = nc.gpsimd.dma_start(
                    out=w[:], in_=ws_d[l - 1, k * P:(k + 1) * P, :])
                ws_dmas.append(d)
                wt[l].append(w)
        bias = None
        if include_bias:
            bias = []
            for l in range(L):
                bt = wpool.tile([1, H], dt.float16, tag=f"b{l}")
                ws_dmas.append(nc.gpsimd.dma_start(out=bt[:],
                                                   in_=bias_d[l, :, :]))
                bias.append(bt)
            ones = wpool.tile([1, TCH], dt.float16, tag="ones")
            nc.gpsimd.memset(ones[:], 1.0)

        # ---- absorber machinery (per-engine pinned chains) ----
        scr_v = wpool.tile([P, 160], dt.float32, tag="scr_v")
        scr_a = wpool.tile([P, 160], dt.float32, tag="scr_a")
        state = {"V": [None, 0], "A": [None, 0], "PE": [None]}

        def absorb(eng, dep=None):
            if eng == "V":
                prev, k = state[eng]
                i = nc.vector.tensor_copy(scr_v[:, k:k + 1], wq[0][0][:])
            elif eng == "A":
                prev, k = state[eng]
                i = nc.scalar.activation(scr_a[:, k:k + 1], wq[0][0][:],
                                         Act.Copy)
            else:
                prev = state[eng][0]
                # 1-column load: cheapest PE instruction that can carry a wait
                i = nc.tensor.ldweights(weights=wt[0][0][:, 0:1])
            if prev is not None:
                bass._add_dep_helper(i.ins, prev.ins, sync=False, reason="chain")
            if dep is not None:
                bass._add_dep_helper(i.ins, dep.ins, sync=True, reason="absorb")
            if eng == "PE":
                state[eng] = [i]
            else:
                state[eng] = [i, (state[eng][1] + 1) % 160]
            return i

        def pin(real, eng):
            prev = state[eng][0]
            if prev is not None:
                bass._add_dep_helper(real.ins, prev.ins, sync=False, reason="pin")
            state[eng][0] = real
            return real

        # warm-up: junk compute while input DMAs land (p-state ramp)
        jw = wpool.tile([P, P], dt.float16, tag="jw")
        jx = wpool.tile([P, 2048], dt.float16, tag="jx")
        jo = wpool.tile([P, 2048], dt.float16, tag="jo")
        jq = wpool.tile([P, 1], dt.float32, tag="jq")
        nc.vector.memset(jw[:, 0:1], 0.125)
        nc.vector.memset(jx[:, 0:1], 0.125)
        nc.vector.memset(jq[:], 0.125)
        for s in range(2):
            jp = psum.tile([P, T], dt.float32, tag="xp")
            for r in range(12):
                pin(nc.tensor.matmul(jp[:, 0:P], lhsT=jw[:],
                                     rhs=jx[:, 0:P],
                                     start=True, stop=True), "PE")
        for r in range(3):
            ji = nc.vector._custom_dve(LSCAN4, out=jo[:], in0=jx[:],
                                       in1=jx[:], s0=jq[:])
            ji.ins.perf_max = 1
            pin(ji, "V")

        # engine init
        for eng in ("V", "A"):
            absorb(eng)
            for d in wq_dmas:
                absorb(eng, d)
        for k in range(KI):
            pin(nc.tensor.ldweights(weights=wt[0][k][:, 0:P]), "PE")
        if include_bias:
            for l in range(L):
                pin(nc.tensor.ldweights(weights=bias[l][:, 0:P]), "PE")
            pin(nc.tensor.ldweights(weights=ones[:, 0:P]), "PE")

        # ---- main loop ----
        NXSB, NHP = 2, 7
        hp_ring = []        # (l, m) keys
        hp_by_key = {}
        hp_readers = {}     # key -> last matmul reading it
        xsb_readers = [None] * NXSB   # ring idx -> last QREC4 reader
        psum_readers = [None, None]   # psum slot -> ACT copy that read it
        xsb_i = 0
        psum_i = 0
        hp_i = 0
        last_qrec = None
        qrec_by = {}
        out_tiles = []  # layer-3 output tiles; fake end-of-trace readers keep
                        # them live so the allocator never hands a DMA-read
                        # buffer to a later tile (a DMA-completion release dep
                        # would exceed walrus's 1-wait budget)

        def mm_rhs(l, k, p, h, n):
            if l == 0:
                return xin[p][k][:, h * 2048 + n * TCH:
                                 h * 2048 + (n + 1) * TCH]
            t0 = h * 1024 + n * 256
            return hp_by_key[(l - 1, k)][:].rearrange(
                "p (t s) -> p t s", s=4)[:, t0:t0 + 256, 2 * p:2 * p + 2]

        def emit_mms(l, m, p, h, xp, ks, stop_k):
            """Emit the k in `ks` matmuls for all 4 chunks of group (p, h).
            stop_k: the k that carries stop (None while pre-issuing)."""
            kprev = KI if l == 0 else M4
            last_mm = None
            for n in range(4):  # 256-timestep chunks
                ns = slice(n * TCH, (n + 1) * TCH)
                if l == 0 and m == 0:
                    c_need = h * 4 + n if p == 0 else h * 2 + n // 2
                    for c, d in xin_dmas[p]:
                        if c == c_need:
                            absorb("PE", d)
                for k in ks:
                    last_mm = nc.tensor.matmul(
                        xp[:, ns], lhsT=wt[l][k][:, m * P:(m + 1) * P],
                        rhs=mm_rhs(l, k, p, h, n), start=(k == 0),
                        stop=(k == stop_k and not include_bias))
                    pin(last_mm, "PE")
                    if l > 0:
                        hp_readers[(l - 1, k)] = last_mm
                if include_bias and stop_k is not None and stop_k in ks:
                    last_mm = pin(nc.tensor.matmul(
                        xp[:, ns], lhsT=bias[l][:, m * P:(m + 1) * P],
                        rhs=ones[:, :], start=False, stop=True), "PE")
            return last_mm

        for l in range(L):
            kprev = KI if l == 0 else M4
            if l > 0:
                for k in range(M4):
                    pin(nc.tensor.ldweights(weights=wt[l][k][:, 0:P]), "PE")
            for m in range(M4):
                xsb = spool.tile([P, QT], dt.float16, tag="xsb", bufs=NXSB)
                xsb_slot = xsb_i % NXSB
                xsb_i += 1
                last_cp = None
                # one matmul computes a batch PAIR's chunk: moving columns
                # are pair-interleaved, so PSUM holds interleaved xp pairs
                # and ACT writes 2-contiguous/skip-2 into the quad tile.
                # At a layer boundary, pre-issue k<last for the first two
                # groups before waiting on the previous layer's final QREC4
                # so PE stays busy (and HAM stays warm) through the handoff.
                HQ = QT // 2
                pre = {}
                if l > 0 and m == 0:
                    # hp(l-1, 0..2) finished long ago; absorbing their QREC4s
                    # lets the k<3 pre-issue run without foreign waits
                    absorb("PE", qrec_by[(l - 1, 2)][1])
                    for h, p in ((0, 0), (0, 1)):
                        xp = psum.tile([P, T], dt.float32, tag="xp")
                        slot = psum_i % 2
                        psum_i += 1
                        old_rd = psum_readers[slot]
                        if old_rd is not None:
                            absorb("PE", old_rd)
                        emit_mms(l, m, p, h, xp, range(kprev - 1), None)
                        pre[(h, p)] = (xp, slot)
                    # h0 finishes only need hp(l-1, 3)'s first half
                    absorb("PE", qrec_by[(l - 1, 3)][0])
                hpt = None
                ls0 = qr0 = None
                for h in range(2):
                    if l > 0 and m == 0 and h == 1:
                        absorb("PE", qrec_by[(l - 1, 3)][1])
                    for p in range(2):
                        if (h, p) in pre:
                            xp, slot = pre[(h, p)]
                            last_mm = emit_mms(l, m, p, h, xp,
                                               [kprev - 1], kprev - 1)
                        else:
                            xp = psum.tile([P, T], dt.float32, tag="xp")
                            slot = psum_i % 2
                            psum_i += 1
                            old_rd = psum_readers[slot]
                            if old_rd is not None:
                                absorb("PE", old_rd)
                            last_mm = emit_mms(l, m, p, h, xp,
                                               range(kprev), kprev - 1)
                        # ACT: PSUM pair-interleaved fp32 -> xsb quad (fp16)
                        old_x = xsb_readers[xsb_slot]
                        if p == 0 and h == 0 and old_x is not None:
                            absorb("A", old_x)
                        absorb("A", last_mm)
                        cp = pin(nc.scalar.activation(
                            xsb[:].rearrange("p (t s) -> p t s", s=4)[
                                :, h * 1024:(h + 1) * 1024, 2 * p:2 * p + 2],
                            xp[:].rearrange("p (t s) -> p t s", s=2),
                            Act.Copy), "A")
                        psum_readers[slot] = cp
                        last_cp = cp
                    # DVE: half-tile scans. h0: seeded LSCAN4; h1: LSCAN4C
                    # continuation, then the QREC pair (the QRECs must not
                    # run between LSCAN4 and LSCAN4C -- shared a-flops)
                    absorb("V", last_cp)
                    if h == 0:
                        ls0 = nc.vector._custom_dve(
                            LSCAN4, out=xsb[:, 0:HQ], in0=xsb[:, 0:HQ],
                            in1=xsb[:, 0:HQ], s0=wq[l][m][:])
                        ls0.ins.perf_max = 1
                        pin(ls0, "V")
                    else:
                        ls1 = nc.vector._custom_dve(
                            LSCAN4C, out=xsb[:, HQ:], in0=xsb[:, HQ:],
                            in1=xsb[:, HQ:], s0=wq[l][m][:])
                        ls1.ins.perf_max = 1
                        pin(ls1, "V")
                        hpt = spool.tile([P, QT], dt.float16, tag="hp",
                                         bufs=NHP)
                        if len(hp_ring) >= NHP:
                            old_key = hp_ring[hp_i % NHP]
                            rd = hp_readers.pop(old_key, None)
                            if rd is not None:
                                absorb("V", rd)
                        if len(hp_ring) < NHP:
                            hp_ring.append((l, m))
                        else:
                            hp_ring[hp_i % NHP] = (l, m)
                        hp_i += 1
                        hp_by_key[(l, m)] = hpt
                        qr0 = nc.vector._custom_dve(
                            QREC4, out=hpt[:, 0:HQ], in0=xsb[:, 0:HQ],
                            in1=xsb[:, 0:HQ], s0=wq[l][m][:])
                        qr0.ins.perf_max = 1
                        pin(qr0, "V")
                        if l == L - 1:
                            nc.sync.dma_start(
                                out=out_d[m * P:(m + 1) * P, 0:HQ],
                                in_=hpt[:, 0:HQ])
                        qr1 = nc.vector._custom_dve(
                            QREC4C, out=hpt[:, HQ:], in0=xsb[:, HQ:],
                            in1=xsb[:, HQ:], s0=wq[l][m][:])
                        qr1.ins.perf_max = 1
                        pin(qr1, "V")
                        xsb_readers[xsb_slot] = qr1
                        last_qrec = qr1
                        qrec_by[(l, m)] = (qr0, qr1)
                        if l == L - 1:
                            nc.sync.dma_start(
                                out=out_d[m * P:(m + 1) * P, HQ:],
                                in_=hpt[:, HQ:])
                            out_tiles.append(hpt)

        # fake readers: pin the DMA'd tiles live until end-of-trace
        for ht in out_tiles:
            pin(nc.vector.tensor_copy(scr_v[:, state["V"][1]:state["V"][1] + 1],
                                      ht[:, 0:1]), "V")
            state["V"][1] = (state["V"][1] + 1) % 160

        # ---- tail pre-drains ----
        tail_deps = [i for i in nc.inst_map.values()
                     if type(i).__name__ == "InstDMACopy"]
        snap = list(nc.inst_map.values())
        for eng in ("DVE", "Activation", "PE"):
            last_e = [i for i in snap
                      if str(getattr(i, "engine", "")).endswith(eng)]
            if last_e:
                tail_deps.append(last_e[-1])
        for depi in tail_deps:
            dr = nc.sync.drain(fusable=False)
            bass._add_dep_helper(dr.ins, depi, sync=True,
                                 reason="tail pre-drain absorber")
    assert mybir.codegen_inst_isa_subclasses(nc)
    _assert_wait_budget(nc)
    return nc


_MULTI_WAIT_OK = {"InstDrain",
                  "InstEventSemaphore", "InstUnconditionalBranch",
                  "InstRegisterMove", "InstISA", "InstTensorLoad",
                  "InstTensorSave"}


def _assert_wait_budget(nc):
    bad = []
    for name, inst in nc.inst_map.items():
        ty = type(inst).__name__
        w = inst.sync_info.on_wait if inst.sync_info else []
        if ty == "InstCustomDveAnt":
            fw = [x for x in w if not x.ant_name.startswith("DVE")]
            if fw:
                bad.append((name, ty, [f"{x.ant_name}>={x.wait_value}"
                                       for x in fw]))
            continue
        if ty in _MULTI_WAIT_OK:
            continue
        if len(w) > 1:
            bad.append((name, ty,
                        [f"{x.ant_name}>={x.wait_value}" for x in w]))
    if bad:
        raise RuntimeError(
            f"{len(bad)} instructions exceed the sync-wait budget, "
            f"first few: {bad[:6]}")


def _prep_core_inputs(Input, W0, Ws, bs, whs, core):
    bsl = slice(core * BLOC, (core + 1) * BLOC)
    xb = Input[bsl]                      # [4, T, I]
    # pair-interleave: xT[p, i, 2t+j] = xb[2p+j, t, i]
    xT = np.ascontiguousarray(
        xb.reshape(2, 2, T, I).transpose(0, 3, 2, 1).reshape(2, I, 2 * T))
    return {
        "xT": xT.astype(np.float16),
        "w0tn": np.ascontiguousarray(-W0.T).astype(np.float16),
        "wstn": np.ascontiguousarray(-Ws.transpose(0, 2, 1)).astype(np.float16),
        "wq": np.ascontiguousarray(
            whs.astype(np.float32).reshape(L, M4, P).transpose(2, 0, 1)
            .reshape(P, L * M4)),
        "biasn": np.ascontiguousarray(-bs[:, None, :]).astype(np.float16),
    }


def kernel(Input, W0, Ws, bs, whs):
    include_bias = bool(np.any(bs != 0))
    nc = build(include_bias=include_bias)
    in_maps = [_prep_core_inputs(Input, W0, Ws, bs, whs, r)
               for r in range(NCORES)]
    res = run_bass_kernel_spmd(nc, in_maps, core_ids=list(range(NCORES)))
    parts = []
    for r in range(NCORES):
        o = res.results[r]["out"]  # [H, 4T] quad-interleaved
        o = o.reshape(H, T, BLOC).transpose(2, 0, 1)  # [BLOC, H, T]
        parts.append(o)
    full = np.concatenate(parts, axis=0)  # [B, H, T]
    return np.ascontiguousarray(full.transpose(0, 2, 1)).astype(np.float32)
